# revision 112
# baseline (speedup 1.0000x reference)
"""DeepseekV2 MLA attention (weight-absorbed, MQA-style latent) on 8 TRN2 NeuronCores.

Sharding: data-parallel over batch (B=2) x tensor-parallel over heads (4 heads/core).
Each core computes, for its batch element and its 4 heads, the partial o_proj
output out_t = [HID, S] (transposed layout); the host sums the 4 partials per
batch element and transposes back.

Precision plan (tolerance 2e-2; measured ~9.8e-3):
  The values path (ckv -> PV -> v_b) stays fp32r, except the hidden/wkv
  projection stream and the v_out/o_proj weights which are bf16 (~0.3% value
  noise).  The attention-score path is fp8 e4m3 with DoubleRow perf mode
  (0.5 cycles/row, 256-deep contraction = 4x fp32r matmul throughput):
  q_nope projection from fp8 hidden/weights (weights pre-scaled x64 out of
  e4m3's subnormal range, compensated in k_b), and q_lat x ckv score
  contractions.  RoPE q/k score operands are bf16.  exp stays fp32.

Structure (per core):
  pass 1 streams bf16 hidden^T once: ckv^T (fp32 psum), k_pe rope, and the
  rope-q projection+rotation (resident bf16 qpr/kper).  Per s-chunk post:
  row-layout RMS stats via ones-matmul + Pool-engine partition_broadcast, fp8
  normed keys ckvT8 [c,(2,2),S]; PE transposes into one PSUM bank per t-tile,
  natural-layout variance via a Square activation's accumulator, one DVE op
  normalizes ckvN [t,c] (ln is folded into v_b host-side).  The chunk-0
  q_nope fp8 DoubleRow projection is hoisted into pass 1's tail (prefetched
  fp8 hidden, resident fp8 weights) to overlap the final post's DVE chain.
  pass 2 per chunk: evacuate q_nope, per head: q_lat (fp32r) quantized into
  fp8 DoubleRow layout, scores = 2 fp8 DoubleRow + 1 bf16 rope matmul per
  128-key tile, max-free exp (magnitudes ~5 std, verified on host), PV +
  ones-matmul rowsum in fp32r, v_b expansion on unnormalized out_lat with a
  single post-normalization (partition_broadcast of the reciprocal rowsum).
  The next chunk's q_nope projection runs between the head loop and o_proj
  so its PSUM banks allocate ahead of oo_ps in the 8-bank FIFO.
"""
import sys

for _p in ("/opt/trn_rl_repo", "/root/.axon_site/_ro/trn_rl_repo"):
    if _p not in sys.path:
        sys.path.insert(0, _p)

import numpy as np

B, S, HID = 2, 2048, 2048
H, DN, DR, KVR, DV = 16, 128, 64, 512, 128
THETA, EPS = 10000.0, 1e-6
SCALE = float((DN + DR) ** -0.5)
NCORES, HL = 8, 4  # 2 (batch) x 4 (head groups of 4)
CH = 512           # s-chunk width (= max fp32 moving operand)


def build_nc(s=S, hid=HID, reps=1):
    import concourse.bacc as bacc
    import concourse.mybir as mybir
    from concourse import tile

    f32 = mybir.dt.float32
    f32r = mybir.dt.float32r
    f8 = mybir.dt.float8e4
    bf16 = mybir.dt.bfloat16
    DR2 = mybir.MatmulPerfMode.DoubleRow
    Exp = mybir.ActivationFunctionType.Exp
    Sqrt = mybir.ActivationFunctionType.Sqrt
    mult = mybir.AluOpType.mult

    def r(ap):
        return ap.bitcast(f32r)

    NCH = s // CH      # s-chunks
    KT = hid // 128    # contraction tiles over HID
    NT = s // 128      # t-tiles

    nc = bacc.Bacc("TRN2", target_bir_lowering=False, debug=False,
                   enable_asserts=False, num_devices=NCORES)

    hid_d = nc.dram_tensor("hid_t", [hid, s], bf16, kind="ExternalInput").ap()
    hid8_d = nc.dram_tensor("hid8_t", [hid, s], f8, kind="ExternalInput").ap()
    wqn8_d = nc.dram_tensor("wqn8_t", [hid, HL * DN], f8, kind="ExternalInput").ap()
    wqp_d = nc.dram_tensor("wqp_t", [hid, 256], bf16, kind="ExternalInput").ap()
    wkv_d = nc.dram_tensor("wkv_t", [hid, KVR + 2 * DR], bf16, kind="ExternalInput").ap()
    ln_d = nc.dram_tensor("ln_t", [128, 4], f32, kind="ExternalInput").ap()
    kb_d = nc.dram_tensor("kb", [HL, DN, KVR], f32r, kind="ExternalInput").ap()
    vb_d = nc.dram_tensor("vb_t", [HL, KVR, DV], f32r, kind="ExternalInput").ap()
    wo_d = nc.dram_tensor("wo_t", [HL * DV, hid], bf16, kind="ExternalInput").ap()
    cos_d = nc.dram_tensor("cos_p", [128, s], f32, kind="ExternalInput").ap()
    sin_d = nc.dram_tensor("sin_p", [128, s], f32, kind="ExternalInput").ap()
    mask_d = nc.dram_tensor("masks", [128, 4, CH], f32, kind="ExternalInput").ap()
    ident_d = nc.dram_tensor("ident", [128, 128], f32r, kind="ExternalInput").ap()
    onec_d = nc.dram_tensor("ones_c", [128, 1], f32r, kind="ExternalInput").ap()
    oner_d = nc.dram_tensor("ones_r", [1, 128], f32r, kind="ExternalInput").ap()
    out_d = nc.dram_tensor("out_t", [hid, s], bf16, kind="ExternalOutput").ap()

    with tile.TileContext(nc) as tc, \
         nc.allow_low_precision(reason="f32r-typed tiles feed fp32r matmuls; psum accum stays fp32"):
        with tc.tile_pool(name="res", bufs=1) as res, \
             tc.tile_pool(name="psp", bufs=8, space="PSUM") as psp:

            def ps_tile(name):
                return psp.tile([128, CH], f32, tag="ps", name=name)

            # resident tiles
            ckvT8 = res.tile([128, 2, 2, s], f8, name="ckvT8")
            wqn8_sb = res.tile([128, KT // 2, 2, HL * DN], f8, name="wqn8_sb")
            wqp_sb = res.tile([128, KT, 256], bf16, name="wqp_sb")
            kper = res.tile([128, s], bf16, name="kper")
            qpr_sb = [res.tile([128, s], bf16, name=f"qpr_sb{p}") for p in range(2)]
            ckvN = [res.tile([128, KVR], f32r, name=f"ckvN{t}") for t in range(NT)]
            kb_sb = res.tile([128, HL, KVR], f32r, name="kb_sb")
            vb_sb = res.tile([128, HL, 4, DV], f32r, name="vb_sb")
            ident = res.tile([128, 128], f32r, name="ident_sb")
            onec = res.tile([128, 1], f32r, name="onec_sb")
            oner = res.tile([1, 128], f32r, name="oner_sb")
            ln_sb = res.tile([128, 4], f32, name="ln_sb")
            ht8_p = res.tile([128, KT // 2, 2, CH], f8, name="ht8_p")  # pass-2 chunk-0 prefetch
            masks = res.tile([128, 4, CH], f32, name="masks_sb")
            sqs_r = [res.tile([128, CH], f32, name=f"sqs_r{i}") for i in range(2)]
            c_r = [res.tile([128, CH], f32r, name=f"c_r{i}") for i in range(8)]
            cos_r = [res.tile([128, CH], f32, name=f"cos_r{i}") for i in range(2)]
            sin_r = [res.tile([128, CH], f32, name=f"sin_r{i}") for i in range(2)]
            varN_r = [res.tile([128, 1], f32, name=f"varN_r{i}") for i in range(2)]
            sdN_r = [res.tile([128, 1], f32, name=f"sdN_r{i}") for i in range(2)]
            ivN_r = [res.tile([128, 1], f32, name=f"ivN_r{i}") for i in range(2)]
            zb128 = res.tile([128, 1], f32, name="zb128")
            epsb = res.tile([1, 1], f32, name="epsb")
            eps128 = res.tile([128, 1], f32, name="eps128")
            nc.vector.memset(zb128[:], 0.0)
            nc.vector.memset(epsb[:], EPS)
            nc.vector.memset(eps128[:], EPS)

            # ---------------- pass 1: latent KV (ckv^T, ckv_nat, k_pe rot) ----
            for _rep in range(reps):
              with tc.tile_pool(name="p1", bufs=1) as p1:
                wkv_sb = p1.tile([128, KT, KVR + 2 * DR], bf16, name="wkv_sb")

                prev_post = None
                for j in range(NCH):
                    sl = slice(j * CH, (j + 1) * CH)
                    # alloc order matters: the "ps" tag is an 8-deep FIFO, so put
                    # the post()-freed banks (qa, kp) first — the next chunk's cps
                    # then 8-back onto early-freeing banks instead
                    qa_ps = [ps_tile(f"qa_ps{p}") for p in range(2)]
                    kp_ps = ps_tile("kp_ps")
                    cps = [ps_tile(f"cps{ci}") for ci in range(4)]
                    for kg in range(KT // 2):
                        ht1 = p1.tile([128, 2, CH], bf16, tag="ht1", bufs=8, name="ht1")
                        nc.sync.dma_start(ht1[:], hid_d[kg * 256:(kg + 1) * 256, sl]
                                          .rearrange("(g p) t -> p g t", p=128))
                        if j == 0:
                            for k0 in (2 * kg, 2 * kg + 1):
                                nc.sync.dma_start(wkv_sb[:, k0, :],
                                                  wkv_d[k0 * 128:(k0 + 1) * 128, :])
                        if j == 0:
                            nc.scalar.dma_start(
                                wqp_sb[:, kg * 2:(kg + 1) * 2, :],
                                wqp_d[kg * 256:(kg + 1) * 256, :]
                                .rearrange("(k p) n -> p k n", p=128))
                            if kg == 0:
                                # small constants, issued behind the first data tiles
                                nc.scalar.dma_start(ident[:], ident_d)
                                nc.scalar.dma_start(onec[:], onec_d)
                                nc.scalar.dma_start(oner[:], oner_d)
                                nc.scalar.dma_start(ln_sb[:], ln_d)
                        # pass-2 weights trickle in sliced, off the critical path
                        if j == 1:
                            nc.scalar.dma_start(
                                wqn8_sb[:, kg, :, :],
                                wqn8_d[kg * 256:(kg + 1) * 256, :]
                                .rearrange("(i p) n -> p i n", p=128))
                        if j == 2 and kg < 4:
                            nc.scalar.dma_start(masks[:, kg, :], mask_d[:, kg, :])
                        # prefetch pass-2 chunk-0 streams during the last chunk
                        if j == 3:
                            nc.gpsimd.dma_start(ht8_p[:, kg], hid8_d[kg * 256:(kg + 1) * 256, 0:CH]
                                                .rearrange("(i p) t -> p i t", p=128))
                        for ki in range(2):
                            k = 2 * kg + ki
                            st_, sp_ = (k == 0), (k == KT - 1)
                            for ci in range(4):
                                nc.tensor.matmul(cps[ci][:], wkv_sb[:, k, ci * 128:(ci + 1) * 128],
                                                 ht1[:, ki, :], start=st_, stop=sp_)
                            nc.tensor.matmul(kp_ps[:], wkv_sb[:, k, KVR:KVR + 128],
                                             ht1[:, ki, :], start=st_, stop=sp_)
                            for p in range(2):
                                nc.tensor.matmul(qa_ps[p][:], wqp_sb[:, k, p * 128:(p + 1) * 128],
                                                 ht1[:, ki, :], start=st_, stop=sp_)

                    # evacuate raw ckv^T to SBUF promptly (frees the 4 cps banks so
                    # the next chunk's k-loop can start); defer the rest of this
                    # chunk's post-processing behind that k-loop.
                    cos1 = cos_r[j % 2]
                    sin1 = sin_r[j % 2]
                    nc.scalar.dma_start(cos1[:], cos_d[:, sl])
                    nc.scalar.dma_start(sin1[:], sin_d[:, sl])

                    c_sb = []
                    for ci in range(4):
                        t = c_r[(j * 4 + ci) % 8]
                        if ci < 2:
                            nc.scalar.copy(t[:], cps[ci][:])
                        else:
                            nc.vector.tensor_copy(t[:], cps[ci][:])
                        c_sb.append(t)
                    # squares computed eagerly so post()'s var matmul is not
                    # blocked on DVE at the next chunk boundary
                    sqt_l = []
                    for ci in range(4):
                        sqt = p1.tile([128, CH], f32r, tag="sqt", bufs=8, name="sqt")
                        if ci % 2 == 0:
                            nc.vector.tensor_mul(sqt[:], c_sb[ci][:], c_sb[ci][:])
                        else:
                            nc.scalar.square(sqt[:], c_sb[ci][:])
                        sqt_l.append(sqt)

                    def make_post(j, sl, c_sb, sqt_l, kp_ps, qa_ps, cos1, sin1):
                        def post():
                            # q_pe rope (moved from pass 2): rotate-half via
                            # sign-flipped cross-partition copies, then combine
                            for p in range(2):
                                qb_sb = p1.tile([128, CH], f32, tag="qb_sb", bufs=2,
                                                name="qb_sb")
                                for base in (0, 64):
                                    nc.vector.tensor_scalar_mul(
                                        qb_sb[base:base + 32, :],
                                        qa_ps[p][base + 32:base + 64, :], -1.0)
                                    nc.vector.tensor_scalar_mul(
                                        qb_sb[base + 32:base + 64, :],
                                        qa_ps[p][base:base + 32, :], 1.0)
                                qr_t = p1.tile([128, CH], f32, tag="qr_t", bufs=2,
                                               name="qr_t")
                                nc.vector.tensor_mul(qr_t[:], qa_ps[p][:], cos1[:])
                                nc.vector.tensor_mul(qb_sb[:], qb_sb[:], sin1[:])
                                nc.vector.tensor_add(qpr_sb[p][:, sl], qr_t[:], qb_sb[:])
                            # RMSNorm over c (partition direction) via ones-matmul
                            var_ps = ps_tile("var_ps")
                            for ci in range(4):
                                sqt = sqt_l[ci]
                                nc.tensor.matmul(var_ps[0:1, :], r(onec[:]), r(sqt[:]),
                                                 start=(ci == 0), stop=(ci == 3))
                            sd1 = p1.tile([1, CH], f32, tag="sd1", bufs=2, name="sd1")
                            nc.scalar.activation(sd1[:], var_ps[0:1, :], Sqrt, bias=epsb[:],
                                                 scale=1.0 / KVR)
                            iv1 = p1.tile([1, CH], f32r, tag="iv1", bufs=2, name="iv1")
                            nc.vector.reciprocal(iv1[:], sd1[:])
                            bc_sb = p1.tile([128, CH], f32r, tag="bc_sb", bufs=2,
                                            name="bc_sb")
                            nc.gpsimd.partition_broadcast(bc_sb[:], iv1[:])
                            for ci in range(4):
                                nc.vector.scalar_tensor_tensor(
                                    ckvT8[:, ci // 2, ci % 2, sl], c_sb[ci][:],
                                    ln_sb[:, ci:ci + 1], bc_sb[:], op0=mult, op1=mult)
                            # k_pe rope: kper = ka*cos + kb*sin (rows 0:64), then duplicate
                            kr_t = p1.tile([128, CH], f32, tag="kr_t", bufs=2, name="kr_t")
                            nc.vector.tensor_mul(kper[0:64, sl], kp_ps[0:64, :], cos1[0:64, :])
                            nc.vector.tensor_mul(kr_t[0:64, :], kp_ps[64:128, :], sin1[0:64, :])
                            nc.vector.tensor_add(kper[0:64, sl], kper[0:64, sl], kr_t[0:64, :])
                            nc.gpsimd.dma_start(kper[64:128, sl], kper[0:64, sl])

                            # transpose raw ckv^T -> natural [t, c] (one PSUM bank per
                            # t-tile), recompute the variance natural-side via a Square
                            # activation's accumulator, normalize with one DVE op.
                            # (ln is folded into v_b host-side)  PE transposes are
                            # batched first so PE frees before the ACT/DVE tail.
                            tn_list = []
                            for ss in range(4):
                                tn_ps = ps_tile("tn_ps")
                                for ci in range(4):
                                    nc.tensor.transpose(r(tn_ps[:, ci * 128:(ci + 1) * 128]),
                                                        c_sb[ci][:, ss * 128:(ss + 1) * 128],
                                                        ident[:])
                                tn_list.append(tn_ps)
                            for ss in range(4):
                                t_i = 4 * j + ss
                                tn_ps = tn_list[ss]
                                # resident scratch: these are the last-released
                                # tiles of the final post; keeping them out of p1
                                # lets the pass-2 pool allocate earlier
                                sqs = sqs_r[ss % 2]
                                varN = varN_r[ss % 2]
                                nc.scalar.activation(sqs[:], tn_ps[:],
                                                     mybir.ActivationFunctionType.Square,
                                                     accum_out=varN[:])
                                sdN = sdN_r[ss % 2]
                                nc.scalar.activation(sdN[:], varN[:], Sqrt, bias=eps128[:],
                                                     scale=1.0 / KVR)
                                ivN = ivN_r[ss % 2]
                                nc.vector.reciprocal(ivN[:], sdN[:])
                                nc.vector.tensor_scalar_mul(ckvN[t_i][:], tn_ps[:], ivN[:])
                        return post

                    post_j = make_post(j, sl, c_sb, sqt_l, kp_ps, qa_ps, cos1, sin1)
                    if prev_post is not None:
                        prev_post()
                    prev_post = post_j

                # chunk-0 q_nope projection (resident/PSUM inputs only) issues
                # ahead of the final chunk's post so PE overlaps its DVE chain
                qn_ps_cur = [ps_tile(f"qn_ps{h}") for h in range(HL)]
                for kg in range(KT // 2):
                    for h in range(HL):
                        nc.tensor.matmul(qn_ps_cur[h][:],
                                         wqn8_sb[:, kg, :, h * 128:(h + 1) * 128],
                                         ht8_p[:, kg], start=(kg == 0),
                                         stop=(kg == KT // 2 - 1), perf_mode=DR2)
                prev_post()

              nc.scalar.dma_start(kb_sb[:], kb_d.rearrange("h d c -> d h c"))
              nc.scalar.dma_start(vb_sb[:], vb_d.rearrange("h (ci p) d -> p h ci d", p=128))

              # ---------------- pass 2: q proj + attention + o_proj -----------
              with tc.tile_pool(name="p2", bufs=1) as p2:
                for j in range(NCH):
                    sl = slice(j * CH, (j + 1) * CH)

                    # evacuate q_nope (projected during the previous chunk, or in
                    # pass 1's tail for chunk 0)
                    qn_sb = []
                    for h in range(HL):
                        t = p2.tile([128, CH], f32r, tag="qn_sb", bufs=4, name=f"qn_sb{h}")
                        nc.scalar.copy(t[:], qn_ps_cur[h][:])
                        qn_sb.append(t)

                    # issue next chunk's fp8 hidden stream with a full chunk of lead
                    if j + 1 < NCH:
                        ht8c = p2.tile([128, KT // 2, 2, CH], f8, tag="ht8c", bufs=2,
                                       name="ht8c")
                        for kg in range(KT // 2):
                            nc.sync.dma_start(
                                ht8c[:, kg],
                                hid8_d[kg * 256:(kg + 1) * 256, (j + 1) * CH:(j + 2) * CH]
                                .rearrange("(i p) t -> p i t", p=128))

                    vo_sb = p2.tile([128, HL, CH], bf16, tag="vo_sb", bufs=1, name="vo_sb")
                    prev_tail = None
                    for h in range(HL):
                        # q_lat^T[c, s] per head, quantized to fp8 DoubleRow layout
                        ql8 = p2.tile([128, 2, 2, CH], f8, tag="ql8", bufs=2, name="ql8")
                        for ci in range(4):
                            ql_ps = ps_tile("ql_ps")
                            nc.tensor.matmul(ql_ps[:], r(kb_sb[:, h, ci * 128:(ci + 1) * 128]),
                                             r(qn_sb[h][:]), start=True, stop=True)
                            if ci % 2 == 0:
                                nc.scalar.copy(ql8[:, ci // 2, ci % 2, :], ql_ps[:])
                            else:
                                nc.vector.tensor_copy(ql8[:, ci // 2, ci % 2, :], ql_ps[:])

                        hp, hh = h // 2, (h % 2) * 64
                        ol_ps = []
                        rs_box = []
                        # t-tile order: diagonal tiles first (first is full-width,
                        # carries start=True), then the off-diagonal history tiles.
                        tts = list(range(4 * j, 4 * j + 4)) + list(range(0, 4 * j))

                        def score_exp(idx):
                            t_i = tts[idx]
                            kd = t_i - 4 * j
                            st = 0 if kd < 0 else (0, 128, 256, 256)[kd]
                            sc_ps = ps_tile("sc_ps")
                            for c2 in range(2):
                                nc.tensor.matmul(sc_ps[:, st:],
                                                 ckvT8[:, c2, :, t_i * 128:(t_i + 1) * 128],
                                                 ql8[:, c2, :, st:], start=(c2 == 0),
                                                 stop=False, perf_mode=DR2)
                            nc.tensor.matmul(sc_ps[:, st:],
                                             kper[hh:hh + 64, t_i * 128:(t_i + 1) * 128],
                                             qpr_sb[hp][hh:hh + 64, j * CH + st:(j + 1) * CH],
                                             start=False, stop=True)
                            if kd >= 0:
                                nc.vector.tensor_add(sc_ps[:, st:], sc_ps[:, st:], masks[:, kd, st:])
                            ex_sb = p2.tile([128, CH], f32r, tag="ex_sb", bufs=4, name="ex_sb")
                            nc.scalar.activation(ex_sb[:, st:], sc_ps[:, st:], Exp,
                                                 bias=zb128[:], scale=SCALE)
                            return ex_sb, st

                        def pv(idx, ex_sb, st):
                            t_i = tts[idx]
                            first, last = (idx == 0), (idx == len(tts) - 1)
                            for ci in range(4):
                                nc.tensor.matmul(ol_ps[ci][:, st:], r(ckvN[t_i][:, ci * 128:(ci + 1) * 128]),
                                                 r(ex_sb[:, st:]), start=first, stop=last)
                            nc.tensor.matmul(rs_box[0][0:1, st:], r(onec[:]), r(ex_sb[:, st:]),
                                             start=first, stop=last)

                        # overlap previous head's tail behind this head's q_lat and
                        # first two score tiles (ol/rs banks allocate only after the
                        # previous head's are released inside prev_tail)
                        npre = min(3, len(tts))
                        pends = [(i,) + score_exp(i) for i in range(npre)]
                        if prev_tail is not None:
                            prev_tail()
                            prev_tail = None
                        ol_ps.extend(ps_tile(f"ol_ps{ci}") for ci in range(4))
                        rs_box.append(ps_tile("rs_ps"))
                        for idx in range(npre, len(tts)):
                            pends.append((idx,) + score_exp(idx))
                            if len(pends) > 5:
                                pv(*pends.pop(0))
                        for pend in pends:
                            pv(*pend)

                        def make_tail(h, ol_ps, rs_ps):
                            def tail():
                                # evacuate unnormalized out_lat (starts right after last PV)
                                ol_sb = p2.tile([128, 4, CH], f32r, tag="ol_sb", bufs=1, name="ol_sb")
                                for ci in range(4):
                                    if ci % 2 == 0:
                                        nc.scalar.copy(ol_sb[:, ci, :], ol_ps[ci][:])
                                    else:
                                        nc.vector.tensor_copy(ol_sb[:, ci, :], ol_ps[ci][:])
                                # softmax denominator -> broadcast tile (parallel chain)
                                rv_sb = p2.tile([1, CH], f32r, tag="rv_sb", bufs=1, name="rv_sb")
                                nc.vector.reciprocal(rv_sb[:], rs_ps[0:1, :])
                                bc2_sb = p2.tile([128, CH], f32r, tag="bc2_sb", bufs=1, name="bc2_sb")
                                nc.gpsimd.partition_broadcast(bc2_sb[:], rv_sb[:])
                                # v_b expansion on unnormalized out_lat; normalize once on
                                # v_out (per-column scaling commutes with the contraction)
                                vo_ps = ps_tile("vo_ps")
                                for ci in range(4):
                                    nc.tensor.matmul(vo_ps[:], r(vb_sb[:, h, ci, :]), r(ol_sb[:, ci, :]),
                                                     start=(ci == 0), stop=(ci == 3))
                                nc.vector.tensor_mul(vo_sb[:, h, :], vo_ps[:], bc2_sb[:])
                            return tail

                        prev_tail = make_tail(h, ol_ps, rs_box[0])
                    prev_tail()

                    # project next chunk's q_nope before o_proj so its PSUM banks
                    # allocate ahead of oo_ps in the bank FIFO
                    if j + 1 < NCH:
                        qn_ps_cur = [ps_tile(f"qn_ps{h}") for h in range(HL)]
                        for kg in range(KT // 2):
                            for h in range(HL):
                                nc.tensor.matmul(qn_ps_cur[h][:],
                                                 wqn8_sb[:, kg, :, h * 128:(h + 1) * 128],
                                                 ht8c[:, kg], start=(kg == 0),
                                                 stop=(kg == KT // 2 - 1), perf_mode=DR2)

                    # o_proj partial: out^T[hid, s] = sum_h wo^T.T @ v_out^T
                    for htile in range(KT):
                        wo_sb = p2.tile([128, HL, 128], bf16, tag="wo_sb", bufs=3, name="wo_sb")
                        nc.sync.dma_start(wo_sb[:], wo_d[:, htile * 128:(htile + 1) * 128]
                                          .rearrange("(a p) n -> p a n", p=128))
                        oo_ps = ps_tile("oo_ps")
                        for hh2 in range(HL):
                            nc.tensor.matmul(oo_ps[:], wo_sb[:, hh2, :], vo_sb[:, hh2, :],
                                             start=(hh2 == 0), stop=(hh2 == HL - 1))
                        oo_sb = p2.tile([128, CH], bf16, tag="oo_sb", bufs=3, name="oo_sb")
                        nc.vector.tensor_copy(oo_sb[:], oo_ps[:])
                        nc.scalar.dma_start(out_d[htile * 128:(htile + 1) * 128, sl], oo_sb[:])

    nc.compile()
    return nc


# ---------------------------------------------------------------------------
# host-side input prep / output assembly
# ---------------------------------------------------------------------------
_PERM = np.concatenate([np.arange(0, DR, 2), np.arange(1, DR, 2)])


def _rope_tables(pos, s):
    inv_freq = 1.0 / (THETA ** (np.arange(0, DR, 2, dtype=np.float64) / DR))
    t = pos.astype(np.float64)
    freqs = t[:, None] * inv_freq
    emb = np.concatenate([freqs, freqs], axis=-1)          # [s, DR]
    cosT = np.cos(emb).T.astype(np.float32)                # [DR, s]
    sinT = np.sin(emb).T.astype(np.float32)
    cos_p = np.ascontiguousarray(np.vstack([cosT, cosT]))  # [128, s]
    sin_p = np.ascontiguousarray(np.vstack([sinT, sinT]))
    return cos_p, sin_p


def _masks():
    t = np.arange(128)[:, None]
    c = np.arange(CH)[None, :]
    m = np.zeros((128, 4, CH), np.float32)
    for kd in range(4):
        m[:, kd, :] = np.where(c >= 128 * kd + t, 0.0, -1e30).astype(np.float32)
    return m


def prep_core_inputs(inputs, core, s=S, hid=HID):
    import ml_dtypes

    b, g = core // 4, core % 4
    heads = slice(HL * g, HL * (g + 1))
    hs = np.asarray(inputs["hidden_states"], np.float32)[b, :s, :hid]
    hid_t = np.ascontiguousarray(hs.T)
    m = {"hid_t": hid_t.astype(ml_dtypes.bfloat16),
         "hid8_t": hid_t.astype(ml_dtypes.float8_e4m3)}

    # scale 0.02-std weights out of e4m3's subnormal range; 1/64 folds into k_b
    wq = np.asarray(inputs["q_nope_weight"], np.float32).reshape(H, DN, HID)[heads, :, :hid]
    wq_t = wq.transpose(2, 0, 1).reshape(hid, HL * DN)
    m["wqn8_t"] = np.ascontiguousarray(wq_t * 64.0).astype(ml_dtypes.float8_e4m3)

    wqp = np.asarray(inputs["q_pe_weight"], np.float32).reshape(H, DR, HID)[heads, :, :hid]
    a = wqp[:, _PERM, :]                                   # [4, 64, hid]
    bv = np.concatenate([-a[:, 32:64], a[:, 0:32]], axis=1)
    A = a.reshape(2, 128, hid)
    Bv = bv.reshape(2, 128, hid)
    wqpe_t = np.concatenate([A[0], A[1]], axis=0).T
    m["wqp_t"] = np.ascontiguousarray(wqpe_t).astype(ml_dtypes.bfloat16)

    wkv = np.asarray(inputs["kv_a_weight"], np.float32)[:, :hid]
    kpe_a = wkv[KVR:][_PERM]
    kpe_b = np.concatenate([-kpe_a[32:], kpe_a[:32]], axis=0)
    m["wkv_t"] = np.ascontiguousarray(
        np.concatenate([wkv[:KVR], kpe_a, kpe_b], axis=0).T).astype(ml_dtypes.bfloat16)

    m["ln_t"] = np.ascontiguousarray(
        np.asarray(inputs["kv_a_ln_weight"], np.float32).reshape(4, 128).T)
    m["kb"] = np.ascontiguousarray(
        np.asarray(inputs["k_b_weight"], np.float32)[heads] / 64.0)
    # ckvN on-device omits the ln weight; fold it into v_b's KVR axis instead
    ln_w = np.asarray(inputs["kv_a_ln_weight"], np.float32)
    m["vb_t"] = np.ascontiguousarray(
        np.asarray(inputs["v_b_weight"], np.float32)[heads].transpose(0, 2, 1)
        * ln_w[None, :, None])
    m["wo_t"] = np.ascontiguousarray(
        np.asarray(inputs["o_weight"], np.float32)[:hid, HL * DV * g:HL * DV * (g + 1)].T
    ).astype(ml_dtypes.bfloat16)

    pos = np.asarray(inputs["position_ids"]).reshape(-1)[:s]
    cos_p, sin_p = _rope_tables(pos, s)
    m["cos_p"], m["sin_p"] = cos_p, sin_p
    m["masks"] = _masks()
    m["ident"] = np.eye(128, dtype=np.float32)
    m["ones_c"] = np.ones((128, 1), np.float32)
    m["ones_r"] = np.ones((1, 128), np.float32)
    return m


_NC_CACHE = {}


def _get_nc():
    if "nc" not in _NC_CACHE:
        _NC_CACHE["nc"] = build_nc()
    return _NC_CACHE["nc"]


def kernel(**inputs):
    from concourse import bass_utils

    nc = _get_nc()
    in_maps = [prep_core_inputs(inputs, c) for c in range(NCORES)]
    res = bass_utils.run_bass_kernel_spmd(nc, in_maps, core_ids=list(range(NCORES)))
    out = np.empty((B, S, HID), np.float32)
    for b in range(B):
        acc = np.array(res.results[4 * b]["out_t"], np.float32)
        for g in range(1, 4):
            acc += res.results[4 * b + g]["out_t"]
        out[b] = acc.T
    return out



# revision 113
# speedup vs baseline: 1.0174x; 1.0174x over previous
"""DeepseekV2 MLA attention (weight-absorbed, MQA-style latent) on 8 TRN2 NeuronCores.

Sharding: data-parallel over batch (B=2) x tensor-parallel over heads (4 heads/core).
Each core computes, for its batch element and its 4 heads, the partial o_proj
output out_t = [HID, S] (transposed layout); the host sums the 4 partials per
batch element and transposes back.

Precision plan (tolerance 2e-2; measured ~9.8e-3):
  The values path (ckv -> PV -> v_b) stays fp32r, except the hidden/wkv
  projection stream and the v_out/o_proj weights which are bf16 (~0.3% value
  noise).  The attention-score path is fp8 e4m3 with DoubleRow perf mode
  (0.5 cycles/row, 256-deep contraction = 4x fp32r matmul throughput):
  q_nope projection from fp8 hidden/weights (weights pre-scaled x64 out of
  e4m3's subnormal range, compensated in k_b), and q_lat x ckv score
  contractions.  RoPE q/k score operands are bf16.  exp stays fp32.

Structure (per core):
  pass 1 streams bf16 hidden^T once: ckv^T (fp32 psum), k_pe rope, and the
  rope-q projection+rotation (resident bf16 qpr/kper).  Per s-chunk post:
  row-layout RMS stats via ones-matmul + Pool-engine partition_broadcast, fp8
  normed keys ckvT8 [c,(2,2),S]; PE transposes into one PSUM bank per t-tile,
  natural-layout variance via a Square activation's accumulator, one DVE op
  normalizes ckvN [t,c] (ln is folded into v_b host-side).  The chunk-0
  q_nope fp8 DoubleRow projection is hoisted into pass 1's tail (prefetched
  fp8 hidden, resident fp8 weights) to overlap the final post's DVE chain.
  pass 2 per chunk: evacuate q_nope, per head: q_lat (fp32r) quantized into
  fp8 DoubleRow layout, scores = 2 fp8 DoubleRow + 1 bf16 rope matmul per
  128-key tile, max-free exp (magnitudes ~5 std, verified on host), PV +
  ones-matmul rowsum in fp32r, v_b expansion on unnormalized out_lat with a
  single post-normalization (partition_broadcast of the reciprocal rowsum).
  The next chunk's q_nope projection runs between the head loop and o_proj
  so its PSUM banks allocate ahead of oo_ps in the 8-bank FIFO.
"""
import sys

for _p in ("/opt/trn_rl_repo", "/root/.axon_site/_ro/trn_rl_repo"):
    if _p not in sys.path:
        sys.path.insert(0, _p)

import numpy as np

B, S, HID = 2, 2048, 2048
H, DN, DR, KVR, DV = 16, 128, 64, 512, 128
THETA, EPS = 10000.0, 1e-6
SCALE = float((DN + DR) ** -0.5)
NCORES, HL = 8, 4  # 2 (batch) x 4 (head groups of 4)
CH = 512           # s-chunk width (= max fp32 moving operand)


def build_nc(s=S, hid=HID, reps=1):
    import concourse.bacc as bacc
    import concourse.mybir as mybir
    from concourse import tile

    f32 = mybir.dt.float32
    f32r = mybir.dt.float32r
    f8 = mybir.dt.float8e4
    bf16 = mybir.dt.bfloat16
    DR2 = mybir.MatmulPerfMode.DoubleRow
    Exp = mybir.ActivationFunctionType.Exp
    Sqrt = mybir.ActivationFunctionType.Sqrt
    mult = mybir.AluOpType.mult

    def r(ap):
        return ap.bitcast(f32r)

    NCH = s // CH      # s-chunks
    KT = hid // 128    # contraction tiles over HID
    NT = s // 128      # t-tiles

    nc = bacc.Bacc("TRN2", target_bir_lowering=False, debug=False,
                   enable_asserts=False, num_devices=NCORES)

    hid_d = nc.dram_tensor("hid_t", [hid, s], bf16, kind="ExternalInput").ap()
    hid8_d = nc.dram_tensor("hid8_t", [hid, s], f8, kind="ExternalInput").ap()
    wqn8_d = nc.dram_tensor("wqn8_t", [hid, HL * DN], f8, kind="ExternalInput").ap()
    wqp_d = nc.dram_tensor("wqp_t", [hid, 256], bf16, kind="ExternalInput").ap()
    wkv_d = nc.dram_tensor("wkv_t", [hid, KVR + 2 * DR], bf16, kind="ExternalInput").ap()
    ln_d = nc.dram_tensor("ln_t", [128, 4], f32, kind="ExternalInput").ap()
    kb_d = nc.dram_tensor("kb", [HL, DN, KVR], f32r, kind="ExternalInput").ap()
    vb_d = nc.dram_tensor("vb_t", [HL, KVR, DV], f32r, kind="ExternalInput").ap()
    wo_d = nc.dram_tensor("wo_t", [HL * DV, hid], bf16, kind="ExternalInput").ap()
    cos_d = nc.dram_tensor("cos_p", [128, s], f32, kind="ExternalInput").ap()
    sin_d = nc.dram_tensor("sin_p", [128, s], f32, kind="ExternalInput").ap()
    mask_d = nc.dram_tensor("masks", [128, 4, CH], f32, kind="ExternalInput").ap()
    ident_d = nc.dram_tensor("ident", [128, 128], f32r, kind="ExternalInput").ap()
    onec_d = nc.dram_tensor("ones_c", [128, 1], f32r, kind="ExternalInput").ap()
    oner_d = nc.dram_tensor("ones_r", [1, 128], f32r, kind="ExternalInput").ap()
    out_d = nc.dram_tensor("out_t", [hid, s], bf16, kind="ExternalOutput").ap()

    with tile.TileContext(nc) as tc, \
         nc.allow_low_precision(reason="f32r-typed tiles feed fp32r matmuls; psum accum stays fp32"):
        with tc.tile_pool(name="res", bufs=1) as res, \
             tc.tile_pool(name="psp", bufs=8, space="PSUM") as psp:

            def ps_tile(name):
                return psp.tile([128, CH], f32, tag="ps", name=name)

            # resident tiles
            ckvT8 = res.tile([128, 2, 2, s], f8, name="ckvT8")
            wqn8_sb = res.tile([128, KT // 2, 2, HL * DN], f8, name="wqn8_sb")
            wqp_sb = res.tile([128, KT, 256], bf16, name="wqp_sb")
            kper = res.tile([128, s], bf16, name="kper")
            qpr_sb = [res.tile([128, s], bf16, name=f"qpr_sb{p}") for p in range(2)]
            ckvN = [res.tile([128, KVR], f32r, name=f"ckvN{t}") for t in range(NT)]
            kb_sb = res.tile([128, HL, KVR], f32r, name="kb_sb")
            vb_sb = res.tile([128, HL, 4, DV], f32r, name="vb_sb")
            ident = res.tile([128, 128], f32r, name="ident_sb")
            onec = res.tile([128, 1], f32r, name="onec_sb")
            oner = res.tile([1, 128], f32r, name="oner_sb")
            ln_sb = res.tile([128, 4], f32, name="ln_sb")
            ht8_p = res.tile([128, KT // 2, 2, CH], f8, name="ht8_p")  # pass-2 chunk-0 prefetch
            masks = res.tile([128, 4, CH], f32, name="masks_sb")
            sqs_r = [res.tile([128, CH], f32, name=f"sqs_r{i}") for i in range(2)]
            c_r = [res.tile([128, CH], f32r, name=f"c_r{i}") for i in range(8)]
            cos_r = [res.tile([128, CH], f32, name=f"cos_r{i}") for i in range(2)]
            sin_r = [res.tile([128, CH], f32, name=f"sin_r{i}") for i in range(2)]
            varN_r = [res.tile([128, 1], f32, name=f"varN_r{i}") for i in range(2)]
            sdN_r = [res.tile([128, 1], f32, name=f"sdN_r{i}") for i in range(2)]
            ivN_r = [res.tile([128, 1], f32, name=f"ivN_r{i}") for i in range(2)]
            zb128 = res.tile([128, 1], f32, name="zb128")
            epsb = res.tile([1, 1], f32, name="epsb")
            eps128 = res.tile([128, 1], f32, name="eps128")
            nc.vector.memset(zb128[:], 0.0)
            nc.vector.memset(epsb[:], EPS)
            nc.vector.memset(eps128[:], EPS)

            # ---------------- pass 1: latent KV (ckv^T, ckv_nat, k_pe rot) ----
            for _rep in range(reps):
              with tc.tile_pool(name="p1", bufs=1) as p1:
                wkv_sb = p1.tile([128, KT, KVR + 2 * DR], bf16, name="wkv_sb")

                prev_post = None
                for j in range(NCH):
                    sl = slice(j * CH, (j + 1) * CH)
                    # alloc order matters: the "ps" tag is an 8-deep FIFO, so put
                    # the post()-freed banks (qa, kp) first — the next chunk's cps
                    # then 8-back onto early-freeing banks instead
                    qa_ps = [ps_tile(f"qa_ps{p}") for p in range(2)]
                    kp_ps = ps_tile("kp_ps")
                    cps = [ps_tile(f"cps{ci}") for ci in range(4)]
                    for kg in range(KT // 2):
                        ht1 = p1.tile([128, 2, CH], bf16, tag="ht1", bufs=8, name="ht1")
                        nc.sync.dma_start(ht1[:], hid_d[kg * 256:(kg + 1) * 256, sl]
                                          .rearrange("(g p) t -> p g t", p=128))
                        if j == 0:
                            for k0 in (2 * kg, 2 * kg + 1):
                                nc.sync.dma_start(wkv_sb[:, k0, :],
                                                  wkv_d[k0 * 128:(k0 + 1) * 128, :])
                        if j == 0:
                            nc.scalar.dma_start(
                                wqp_sb[:, kg * 2:(kg + 1) * 2, :],
                                wqp_d[kg * 256:(kg + 1) * 256, :]
                                .rearrange("(k p) n -> p k n", p=128))
                            if kg == 0:
                                # small constants, issued behind the first data tiles
                                nc.scalar.dma_start(ident[:], ident_d)
                                nc.scalar.dma_start(onec[:], onec_d)
                                nc.scalar.dma_start(oner[:], oner_d)
                                nc.scalar.dma_start(ln_sb[:], ln_d)
                        # pass-2 weights trickle in sliced, off the critical path
                        if j == 1:
                            nc.scalar.dma_start(
                                wqn8_sb[:, kg, :, :],
                                wqn8_d[kg * 256:(kg + 1) * 256, :]
                                .rearrange("(i p) n -> p i n", p=128))
                        if j == 2 and kg < 4:
                            nc.scalar.dma_start(masks[:, kg, :], mask_d[:, kg, :])
                        # prefetch pass-2 chunk-0 streams during the last chunk
                        if j == 3:
                            nc.gpsimd.dma_start(ht8_p[:, kg], hid8_d[kg * 256:(kg + 1) * 256, 0:CH]
                                                .rearrange("(i p) t -> p i t", p=128))
                        for ki in range(2):
                            k = 2 * kg + ki
                            st_, sp_ = (k == 0), (k == KT - 1)
                            for ci in range(4):
                                nc.tensor.matmul(cps[ci][:], wkv_sb[:, k, ci * 128:(ci + 1) * 128],
                                                 ht1[:, ki, :], start=st_, stop=sp_)
                            nc.tensor.matmul(kp_ps[:], wkv_sb[:, k, KVR:KVR + 128],
                                             ht1[:, ki, :], start=st_, stop=sp_)
                            for p in range(2):
                                nc.tensor.matmul(qa_ps[p][:], wqp_sb[:, k, p * 128:(p + 1) * 128],
                                                 ht1[:, ki, :], start=st_, stop=sp_)

                    # evacuate raw ckv^T to SBUF promptly (frees the 4 cps banks so
                    # the next chunk's k-loop can start); defer the rest of this
                    # chunk's post-processing behind that k-loop.
                    cos1 = cos_r[j % 2]
                    sin1 = sin_r[j % 2]
                    nc.scalar.dma_start(cos1[:], cos_d[:, sl])
                    nc.scalar.dma_start(sin1[:], sin_d[:, sl])

                    c_sb = []
                    for ci in range(4):
                        t = c_r[(j * 4 + ci) % 8]
                        if ci < 2:
                            nc.scalar.copy(t[:], cps[ci][:])
                        else:
                            nc.vector.tensor_copy(t[:], cps[ci][:])
                        c_sb.append(t)
                    # squares computed eagerly so post()'s var matmul is not
                    # blocked on DVE at the next chunk boundary
                    sqt_l = []
                    for ci in range(4):
                        sqt = p1.tile([128, CH], f32r, tag="sqt", bufs=8, name="sqt")
                        if ci % 2 == 0:
                            nc.vector.tensor_mul(sqt[:], c_sb[ci][:], c_sb[ci][:])
                        else:
                            nc.scalar.square(sqt[:], c_sb[ci][:])
                        sqt_l.append(sqt)

                    def make_post(j, sl, c_sb, sqt_l, kp_ps, qa_ps, cos1, sin1):
                        def post():
                            # q_pe rope (moved from pass 2): rotate-half via
                            # sign-flipped cross-partition copies, then combine
                            for p in range(2):
                                qb_sb = p1.tile([128, CH], f32, tag="qb_sb", bufs=2,
                                                name="qb_sb")
                                for base in (0, 64):
                                    nc.vector.tensor_scalar_mul(
                                        qb_sb[base:base + 32, :],
                                        qa_ps[p][base + 32:base + 64, :], -1.0)
                                    nc.vector.tensor_scalar_mul(
                                        qb_sb[base + 32:base + 64, :],
                                        qa_ps[p][base:base + 32, :], 1.0)
                                qr_t = p1.tile([128, CH], f32, tag="qr_t", bufs=2,
                                               name="qr_t")
                                nc.vector.tensor_mul(qr_t[:], qa_ps[p][:], cos1[:])
                                nc.vector.tensor_mul(qb_sb[:], qb_sb[:], sin1[:])
                                nc.vector.tensor_add(qpr_sb[p][:, sl], qr_t[:], qb_sb[:])
                            # RMSNorm over c (partition direction) via ones-matmul
                            var_ps = ps_tile("var_ps")
                            for ci in range(4):
                                sqt = sqt_l[ci]
                                nc.tensor.matmul(var_ps[0:1, :], r(onec[:]), r(sqt[:]),
                                                 start=(ci == 0), stop=(ci == 3))
                            sd1 = p1.tile([1, CH], f32, tag="sd1", bufs=2, name="sd1")
                            nc.scalar.activation(sd1[:], var_ps[0:1, :], Sqrt, bias=epsb[:],
                                                 scale=1.0 / KVR)
                            iv1 = p1.tile([1, CH], f32r, tag="iv1", bufs=2, name="iv1")
                            nc.vector.reciprocal(iv1[:], sd1[:])
                            bc_sb = p1.tile([128, CH], f32r, tag="bc_sb", bufs=2,
                                            name="bc_sb")
                            nc.gpsimd.partition_broadcast(bc_sb[:], iv1[:])
                            for ci in range(4):
                                nc.vector.scalar_tensor_tensor(
                                    ckvT8[:, ci // 2, ci % 2, sl], c_sb[ci][:],
                                    ln_sb[:, ci:ci + 1], bc_sb[:], op0=mult, op1=mult)
                            # k_pe rope: kper = ka*cos + kb*sin (rows 0:64), then duplicate
                            kr_t = p1.tile([128, CH], f32, tag="kr_t", bufs=2, name="kr_t")
                            nc.vector.tensor_mul(kper[0:64, sl], kp_ps[0:64, :], cos1[0:64, :])
                            nc.vector.tensor_mul(kr_t[0:64, :], kp_ps[64:128, :], sin1[0:64, :])
                            nc.vector.tensor_add(kper[0:64, sl], kper[0:64, sl], kr_t[0:64, :])
                            nc.gpsimd.dma_start(kper[64:128, sl], kper[0:64, sl])

                            # transpose raw ckv^T -> natural [t, c] (one PSUM bank per
                            # t-tile), recompute the variance natural-side via a Square
                            # activation's accumulator, normalize with one DVE op.
                            # (ln is folded into v_b host-side)  PE transposes are
                            # batched first so PE frees before the ACT/DVE tail.
                            tn_list = []
                            for ss in range(4):
                                tn_ps = ps_tile("tn_ps")
                                for ci in range(4):
                                    nc.tensor.transpose(r(tn_ps[:, ci * 128:(ci + 1) * 128]),
                                                        c_sb[ci][:, ss * 128:(ss + 1) * 128],
                                                        ident[:])
                                tn_list.append(tn_ps)
                            for ss in range(4):
                                t_i = 4 * j + ss
                                tn_ps = tn_list[ss]
                                # resident scratch: these are the last-released
                                # tiles of the final post; keeping them out of p1
                                # lets the pass-2 pool allocate earlier
                                sqs = sqs_r[ss % 2]
                                varN = varN_r[ss % 2]
                                nc.scalar.activation(sqs[:], tn_ps[:],
                                                     mybir.ActivationFunctionType.Square,
                                                     accum_out=varN[:])
                                sdN = sdN_r[ss % 2]
                                nc.scalar.activation(sdN[:], varN[:], Sqrt, bias=eps128[:],
                                                     scale=1.0 / KVR)
                                ivN = ivN_r[ss % 2]
                                nc.vector.reciprocal(ivN[:], sdN[:])
                                nc.vector.tensor_scalar_mul(ckvN[t_i][:], tn_ps[:], ivN[:])
                        return post

                    post_j = make_post(j, sl, c_sb, sqt_l, kp_ps, qa_ps, cos1, sin1)
                    if prev_post is not None:
                        prev_post()
                    prev_post = post_j

                # chunk-0 q_nope projection (resident/PSUM inputs only) issues
                # ahead of the final chunk's post so PE overlaps its DVE chain
                qn_ps_cur = [ps_tile(f"qn_ps{h}") for h in range(HL)]
                for kg in range(KT // 2):
                    for h in range(HL):
                        nc.tensor.matmul(qn_ps_cur[h][:],
                                         wqn8_sb[:, kg, :, h * 128:(h + 1) * 128],
                                         ht8_p[:, kg], start=(kg == 0),
                                         stop=(kg == KT // 2 - 1), perf_mode=DR2)
                prev_post()

              nc.scalar.dma_start(kb_sb[:], kb_d.rearrange("h d c -> d h c"))
              nc.scalar.dma_start(vb_sb[:], vb_d.rearrange("h (ci p) d -> p h ci d", p=128))

              # ---------------- pass 2: q proj + attention + o_proj -----------
              with tc.tile_pool(name="p2", bufs=1) as p2:
                for j in range(NCH):
                    sl = slice(j * CH, (j + 1) * CH)

                    # evacuate q_nope (projected during the previous chunk, or in
                    # pass 1's tail for chunk 0)
                    qn_sb = []
                    for h in range(HL):
                        t = p2.tile([128, CH], f32r, tag="qn_sb", bufs=4, name=f"qn_sb{h}")
                        nc.scalar.copy(t[:], qn_ps_cur[h][:])
                        qn_sb.append(t)

                    # issue next chunk's fp8 hidden stream with a full chunk of lead
                    if j + 1 < NCH:
                        ht8c = p2.tile([128, KT // 2, 2, CH], f8, tag="ht8c", bufs=2,
                                       name="ht8c")
                        for kg in range(KT // 2):
                            nc.sync.dma_start(
                                ht8c[:, kg],
                                hid8_d[kg * 256:(kg + 1) * 256, (j + 1) * CH:(j + 2) * CH]
                                .rearrange("(i p) t -> p i t", p=128))

                    vo_sb = p2.tile([128, HL, CH], bf16, tag="vo_sb", bufs=1, name="vo_sb")
                    prev_tail = None
                    for h in range(HL):
                        # q_lat^T[c, s] per head, quantized to fp8 DoubleRow layout
                        ql8 = p2.tile([128, 2, 2, CH], f8, tag="ql8", bufs=2, name="ql8")
                        for ci in range(4):
                            ql_ps = ps_tile("ql_ps")
                            nc.tensor.matmul(ql_ps[:], r(kb_sb[:, h, ci * 128:(ci + 1) * 128]),
                                             r(qn_sb[h][:]), start=True, stop=True)
                            if ci % 2 == 0:
                                nc.scalar.copy(ql8[:, ci // 2, ci % 2, :], ql_ps[:])
                            else:
                                nc.vector.tensor_copy(ql8[:, ci // 2, ci % 2, :], ql_ps[:])

                        hp, hh = h // 2, (h % 2) * 64
                        ol_ps = []
                        rs_box = []
                        # t-tile order: diagonal tiles first (first is full-width,
                        # carries start=True), then the off-diagonal history tiles.
                        tts = list(range(4 * j, 4 * j + 4)) + list(range(0, 4 * j))

                        def score_exp(idx):
                            t_i = tts[idx]
                            kd = t_i - 4 * j
                            st = 0 if kd < 0 else (0, 128, 256, 256)[kd]
                            sc_ps = ps_tile("sc_ps")
                            for c2 in range(2):
                                nc.tensor.matmul(sc_ps[:, st:],
                                                 ckvT8[:, c2, :, t_i * 128:(t_i + 1) * 128],
                                                 ql8[:, c2, :, st:], start=(c2 == 0),
                                                 stop=False, perf_mode=DR2)
                            nc.tensor.matmul(sc_ps[:, st:],
                                             kper[hh:hh + 64, t_i * 128:(t_i + 1) * 128],
                                             qpr_sb[hp][hh:hh + 64, j * CH + st:(j + 1) * CH],
                                             start=False, stop=True)
                            if kd >= 0:
                                nc.vector.tensor_add(sc_ps[:, st:], sc_ps[:, st:], masks[:, kd, st:])
                            ex_sb = p2.tile([128, CH], f32r, tag="ex_sb", bufs=4, name="ex_sb")
                            nc.scalar.activation(ex_sb[:, st:], sc_ps[:, st:], Exp,
                                                 bias=zb128[:], scale=SCALE)
                            return ex_sb, st

                        def pv(idx, ex_sb, st):
                            t_i = tts[idx]
                            first, last = (idx == 0), (idx == len(tts) - 1)
                            for ci in range(4):
                                nc.tensor.matmul(ol_ps[ci][:, st:], r(ckvN[t_i][:, ci * 128:(ci + 1) * 128]),
                                                 r(ex_sb[:, st:]), start=first, stop=last)
                            nc.tensor.matmul(rs_box[0][0:1, st:], r(onec[:]), r(ex_sb[:, st:]),
                                             start=first, stop=last)

                        # overlap previous head's tail behind this head's q_lat and
                        # first two score tiles (ol/rs banks allocate only after the
                        # previous head's are released inside prev_tail)
                        npre = min(3, len(tts))
                        pends = [(i,) + score_exp(i) for i in range(npre)]
                        if prev_tail is not None:
                            prev_tail()
                            prev_tail = None
                        ol_ps.extend(ps_tile(f"ol_ps{ci}") for ci in range(4))
                        rs_box.append(ps_tile("rs_ps"))
                        for idx in range(npre, len(tts)):
                            pends.append((idx,) + score_exp(idx))
                            if len(pends) > 5:
                                pv(*pends.pop(0))
                        for pend in pends:
                            pv(*pend)

                        def make_tail(h, ol_ps, rs_ps):
                            def tail():
                                # evacuate unnormalized out_lat (starts right after last PV)
                                ol_sb = p2.tile([128, 4, CH], f32r, tag="ol_sb", bufs=1, name="ol_sb")
                                for ci in range(4):
                                    if ci % 2 == 0:
                                        nc.scalar.copy(ol_sb[:, ci, :], ol_ps[ci][:])
                                    else:
                                        nc.vector.tensor_copy(ol_sb[:, ci, :], ol_ps[ci][:])
                                # softmax denominator -> broadcast tile (parallel chain)
                                rv_sb = p2.tile([1, CH], f32r, tag="rv_sb", bufs=1, name="rv_sb")
                                nc.vector.reciprocal(rv_sb[:], rs_ps[0:1, :])
                                bc2_sb = p2.tile([128, CH], f32r, tag="bc2_sb", bufs=1, name="bc2_sb")
                                nc.gpsimd.partition_broadcast(bc2_sb[:], rv_sb[:])
                                # v_b expansion on unnormalized out_lat; normalize once on
                                # v_out (per-column scaling commutes with the contraction)
                                vo_ps = ps_tile("vo_ps")
                                for ci in range(4):
                                    nc.tensor.matmul(vo_ps[:], r(vb_sb[:, h, ci, :]), r(ol_sb[:, ci, :]),
                                                     start=(ci == 0), stop=(ci == 3))
                                nc.vector.tensor_mul(vo_sb[:, h, :], vo_ps[:], bc2_sb[:])
                            return tail

                        prev_tail = make_tail(h, ol_ps, rs_box[0])
                    prev_tail()

                    # project next chunk's q_nope before o_proj so its PSUM banks
                    # allocate ahead of oo_ps in the bank FIFO
                    if j + 1 < NCH:
                        qn_ps_cur = [ps_tile(f"qn_ps{h}") for h in range(HL)]
                        for kg in range(KT // 2):
                            for h in range(HL):
                                nc.tensor.matmul(qn_ps_cur[h][:],
                                                 wqn8_sb[:, kg, :, h * 128:(h + 1) * 128],
                                                 ht8c[:, kg], start=(kg == 0),
                                                 stop=(kg == KT // 2 - 1), perf_mode=DR2)

                    # o_proj partial: out^T[hid, s] = sum_h wo^T.T @ v_out^T
                    for htile in range(KT):
                        wo_sb = p2.tile([128, HL, 128], bf16, tag="wo_sb", bufs=4, name="wo_sb")
                        nc.sync.dma_start(wo_sb[:], wo_d[:, htile * 128:(htile + 1) * 128]
                                          .rearrange("(a p) n -> p a n", p=128))
                        oo_ps = ps_tile("oo_ps")
                        for hh2 in range(HL):
                            nc.tensor.matmul(oo_ps[:], wo_sb[:, hh2, :], vo_sb[:, hh2, :],
                                             start=(hh2 == 0), stop=(hh2 == HL - 1))
                        oo_sb = p2.tile([128, CH], bf16, tag="oo_sb", bufs=3, name="oo_sb")
                        nc.vector.tensor_copy(oo_sb[:], oo_ps[:])
                        nc.scalar.dma_start(out_d[htile * 128:(htile + 1) * 128, sl], oo_sb[:])

    nc.compile()
    return nc


# ---------------------------------------------------------------------------
# host-side input prep / output assembly
# ---------------------------------------------------------------------------
_PERM = np.concatenate([np.arange(0, DR, 2), np.arange(1, DR, 2)])


def _rope_tables(pos, s):
    inv_freq = 1.0 / (THETA ** (np.arange(0, DR, 2, dtype=np.float64) / DR))
    t = pos.astype(np.float64)
    freqs = t[:, None] * inv_freq
    emb = np.concatenate([freqs, freqs], axis=-1)          # [s, DR]
    cosT = np.cos(emb).T.astype(np.float32)                # [DR, s]
    sinT = np.sin(emb).T.astype(np.float32)
    cos_p = np.ascontiguousarray(np.vstack([cosT, cosT]))  # [128, s]
    sin_p = np.ascontiguousarray(np.vstack([sinT, sinT]))
    return cos_p, sin_p


def _masks():
    t = np.arange(128)[:, None]
    c = np.arange(CH)[None, :]
    m = np.zeros((128, 4, CH), np.float32)
    for kd in range(4):
        m[:, kd, :] = np.where(c >= 128 * kd + t, 0.0, -1e30).astype(np.float32)
    return m


def prep_core_inputs(inputs, core, s=S, hid=HID):
    import ml_dtypes

    b, g = core // 4, core % 4
    heads = slice(HL * g, HL * (g + 1))
    hs = np.asarray(inputs["hidden_states"], np.float32)[b, :s, :hid]
    hid_t = np.ascontiguousarray(hs.T)
    m = {"hid_t": hid_t.astype(ml_dtypes.bfloat16),
         "hid8_t": hid_t.astype(ml_dtypes.float8_e4m3)}

    # scale 0.02-std weights out of e4m3's subnormal range; 1/64 folds into k_b
    wq = np.asarray(inputs["q_nope_weight"], np.float32).reshape(H, DN, HID)[heads, :, :hid]
    wq_t = wq.transpose(2, 0, 1).reshape(hid, HL * DN)
    m["wqn8_t"] = np.ascontiguousarray(wq_t * 64.0).astype(ml_dtypes.float8_e4m3)

    wqp = np.asarray(inputs["q_pe_weight"], np.float32).reshape(H, DR, HID)[heads, :, :hid]
    a = wqp[:, _PERM, :]                                   # [4, 64, hid]
    bv = np.concatenate([-a[:, 32:64], a[:, 0:32]], axis=1)
    A = a.reshape(2, 128, hid)
    Bv = bv.reshape(2, 128, hid)
    wqpe_t = np.concatenate([A[0], A[1]], axis=0).T
    m["wqp_t"] = np.ascontiguousarray(wqpe_t).astype(ml_dtypes.bfloat16)

    wkv = np.asarray(inputs["kv_a_weight"], np.float32)[:, :hid]
    kpe_a = wkv[KVR:][_PERM]
    kpe_b = np.concatenate([-kpe_a[32:], kpe_a[:32]], axis=0)
    m["wkv_t"] = np.ascontiguousarray(
        np.concatenate([wkv[:KVR], kpe_a, kpe_b], axis=0).T).astype(ml_dtypes.bfloat16)

    m["ln_t"] = np.ascontiguousarray(
        np.asarray(inputs["kv_a_ln_weight"], np.float32).reshape(4, 128).T)
    m["kb"] = np.ascontiguousarray(
        np.asarray(inputs["k_b_weight"], np.float32)[heads] / 64.0)
    # ckvN on-device omits the ln weight; fold it into v_b's KVR axis instead
    ln_w = np.asarray(inputs["kv_a_ln_weight"], np.float32)
    m["vb_t"] = np.ascontiguousarray(
        np.asarray(inputs["v_b_weight"], np.float32)[heads].transpose(0, 2, 1)
        * ln_w[None, :, None])
    m["wo_t"] = np.ascontiguousarray(
        np.asarray(inputs["o_weight"], np.float32)[:hid, HL * DV * g:HL * DV * (g + 1)].T
    ).astype(ml_dtypes.bfloat16)

    pos = np.asarray(inputs["position_ids"]).reshape(-1)[:s]
    cos_p, sin_p = _rope_tables(pos, s)
    m["cos_p"], m["sin_p"] = cos_p, sin_p
    m["masks"] = _masks()
    m["ident"] = np.eye(128, dtype=np.float32)
    m["ones_c"] = np.ones((128, 1), np.float32)
    m["ones_r"] = np.ones((1, 128), np.float32)
    return m


_NC_CACHE = {}


def _get_nc():
    if "nc" not in _NC_CACHE:
        _NC_CACHE["nc"] = build_nc()
    return _NC_CACHE["nc"]


def kernel(**inputs):
    from concourse import bass_utils

    nc = _get_nc()
    in_maps = [prep_core_inputs(inputs, c) for c in range(NCORES)]
    res = bass_utils.run_bass_kernel_spmd(nc, in_maps, core_ids=list(range(NCORES)))
    out = np.empty((B, S, HID), np.float32)
    for b in range(B):
        acc = np.array(res.results[4 * b]["out_t"], np.float32)
        for g in range(1, 4):
            acc += res.results[4 * b + g]["out_t"]
        out[b] = acc.T
    return out



# revision 114
# speedup vs baseline: 1.0198x; 1.0023x over previous
"""DeepseekV2 MLA attention (weight-absorbed, MQA-style latent) on 8 TRN2 NeuronCores.

Sharding: data-parallel over batch (B=2) x tensor-parallel over heads (4 heads/core).
Each core computes, for its batch element and its 4 heads, the partial o_proj
output out_t = [HID, S] (transposed layout); the host sums the 4 partials per
batch element and transposes back.

Precision plan (tolerance 2e-2; measured ~9.8e-3):
  The values path (ckv -> PV -> v_b) stays fp32r, except the hidden/wkv
  projection stream and the v_out/o_proj weights which are bf16 (~0.3% value
  noise).  The attention-score path is fp8 e4m3 with DoubleRow perf mode
  (0.5 cycles/row, 256-deep contraction = 4x fp32r matmul throughput):
  q_nope projection from fp8 hidden/weights (weights pre-scaled x64 out of
  e4m3's subnormal range, compensated in k_b), and q_lat x ckv score
  contractions.  RoPE q/k score operands are bf16.  exp stays fp32.

Structure (per core):
  pass 1 streams bf16 hidden^T once: ckv^T (fp32 psum), k_pe rope, and the
  rope-q projection+rotation (resident bf16 qpr/kper).  Per s-chunk post:
  row-layout RMS stats via ones-matmul + Pool-engine partition_broadcast, fp8
  normed keys ckvT8 [c,(2,2),S]; PE transposes into one PSUM bank per t-tile,
  natural-layout variance via a Square activation's accumulator, one DVE op
  normalizes ckvN [t,c] (ln is folded into v_b host-side).  The chunk-0
  q_nope fp8 DoubleRow projection is hoisted into pass 1's tail (prefetched
  fp8 hidden, resident fp8 weights) to overlap the final post's DVE chain.
  pass 2 per chunk: evacuate q_nope, per head: q_lat (fp32r) quantized into
  fp8 DoubleRow layout, scores = 2 fp8 DoubleRow + 1 bf16 rope matmul per
  128-key tile, max-free exp (magnitudes ~5 std, verified on host), PV +
  ones-matmul rowsum in fp32r, v_b expansion on unnormalized out_lat with a
  single post-normalization (partition_broadcast of the reciprocal rowsum).
  The next chunk's q_nope projection runs between the head loop and o_proj
  so its PSUM banks allocate ahead of oo_ps in the 8-bank FIFO.
"""
import sys

for _p in ("/opt/trn_rl_repo", "/root/.axon_site/_ro/trn_rl_repo"):
    if _p not in sys.path:
        sys.path.insert(0, _p)

import numpy as np

B, S, HID = 2, 2048, 2048
H, DN, DR, KVR, DV = 16, 128, 64, 512, 128
THETA, EPS = 10000.0, 1e-6
SCALE = float((DN + DR) ** -0.5)
NCORES, HL = 8, 4  # 2 (batch) x 4 (head groups of 4)
CH = 512           # s-chunk width (= max fp32 moving operand)


def build_nc(s=S, hid=HID, reps=1):
    import concourse.bacc as bacc
    import concourse.mybir as mybir
    from concourse import tile

    f32 = mybir.dt.float32
    f32r = mybir.dt.float32r
    f8 = mybir.dt.float8e4
    bf16 = mybir.dt.bfloat16
    DR2 = mybir.MatmulPerfMode.DoubleRow
    Exp = mybir.ActivationFunctionType.Exp
    Sqrt = mybir.ActivationFunctionType.Sqrt
    mult = mybir.AluOpType.mult

    def r(ap):
        return ap.bitcast(f32r)

    NCH = s // CH      # s-chunks
    KT = hid // 128    # contraction tiles over HID
    NT = s // 128      # t-tiles

    nc = bacc.Bacc("TRN2", target_bir_lowering=False, debug=False,
                   enable_asserts=False, num_devices=NCORES)

    hid_d = nc.dram_tensor("hid_t", [hid, s], bf16, kind="ExternalInput").ap()
    hid8_d = nc.dram_tensor("hid8_t", [hid, s], f8, kind="ExternalInput").ap()
    wqn8_d = nc.dram_tensor("wqn8_t", [hid, HL * DN], f8, kind="ExternalInput").ap()
    wqp_d = nc.dram_tensor("wqp_t", [hid, 256], bf16, kind="ExternalInput").ap()
    wkv_d = nc.dram_tensor("wkv_t", [hid, KVR + 2 * DR], bf16, kind="ExternalInput").ap()
    ln_d = nc.dram_tensor("ln_t", [128, 4], f32, kind="ExternalInput").ap()
    kb_d = nc.dram_tensor("kb", [HL, DN, KVR], f32r, kind="ExternalInput").ap()
    vb_d = nc.dram_tensor("vb_t", [HL, KVR, DV], f32r, kind="ExternalInput").ap()
    wo_d = nc.dram_tensor("wo_t", [HL * DV, hid], bf16, kind="ExternalInput").ap()
    cos_d = nc.dram_tensor("cos_p", [128, s], f32, kind="ExternalInput").ap()
    sin_d = nc.dram_tensor("sin_p", [128, s], f32, kind="ExternalInput").ap()
    mask_d = nc.dram_tensor("masks", [128, 4, CH], f32, kind="ExternalInput").ap()
    ident_d = nc.dram_tensor("ident", [128, 128], f32r, kind="ExternalInput").ap()
    onec_d = nc.dram_tensor("ones_c", [128, 1], f32r, kind="ExternalInput").ap()
    oner_d = nc.dram_tensor("ones_r", [1, 128], f32r, kind="ExternalInput").ap()
    out_d = nc.dram_tensor("out_t", [hid, s], bf16, kind="ExternalOutput").ap()

    with tile.TileContext(nc) as tc, \
         nc.allow_low_precision(reason="f32r-typed tiles feed fp32r matmuls; psum accum stays fp32"):
        with tc.tile_pool(name="res", bufs=1) as res, \
             tc.tile_pool(name="psp", bufs=8, space="PSUM") as psp:

            def ps_tile(name):
                return psp.tile([128, CH], f32, tag="ps", name=name)

            # resident tiles
            ckvT8 = res.tile([128, 2, 2, s], f8, name="ckvT8")
            wqn8_sb = res.tile([128, KT // 2, 2, HL * DN], f8, name="wqn8_sb")
            wqp_sb = res.tile([128, KT, 256], bf16, name="wqp_sb")
            kper = res.tile([128, s], bf16, name="kper")
            qpr_sb = [res.tile([128, s], bf16, name=f"qpr_sb{p}") for p in range(2)]
            ckvN = [res.tile([128, KVR], f32r, name=f"ckvN{t}") for t in range(NT)]
            kb_sb = res.tile([128, HL, KVR], f32r, name="kb_sb")
            vb_sb = res.tile([128, HL, 4, DV], f32r, name="vb_sb")
            ident = res.tile([128, 128], f32r, name="ident_sb")
            onec = res.tile([128, 1], f32r, name="onec_sb")
            oner = res.tile([1, 128], f32r, name="oner_sb")
            ln_sb = res.tile([128, 4], f32, name="ln_sb")
            ht8_p = res.tile([128, KT // 2, 2, CH], f8, name="ht8_p")  # pass-2 chunk-0 prefetch
            masks = res.tile([128, 4, CH], f32, name="masks_sb")
            sqs_r = [res.tile([128, CH], f32, name=f"sqs_r{i}") for i in range(2)]
            c_r = [res.tile([128, CH], f32r, name=f"c_r{i}") for i in range(8)]
            cos_r = [res.tile([128, CH], f32, name=f"cos_r{i}") for i in range(2)]
            sin_r = [res.tile([128, CH], f32, name=f"sin_r{i}") for i in range(2)]
            varN_r = [res.tile([128, 1], f32, name=f"varN_r{i}") for i in range(2)]
            sdN_r = [res.tile([128, 1], f32, name=f"sdN_r{i}") for i in range(2)]
            ivN_r = [res.tile([128, 1], f32, name=f"ivN_r{i}") for i in range(2)]
            zb128 = res.tile([128, 1], f32, name="zb128")
            epsb = res.tile([1, 1], f32, name="epsb")
            eps128 = res.tile([128, 1], f32, name="eps128")
            nc.vector.memset(zb128[:], 0.0)
            nc.vector.memset(epsb[:], EPS)
            nc.vector.memset(eps128[:], EPS)

            # ---------------- pass 1: latent KV (ckv^T, ckv_nat, k_pe rot) ----
            for _rep in range(reps):
              with tc.tile_pool(name="p1", bufs=1) as p1:
                wkv_sb = p1.tile([128, KT, KVR + 2 * DR], bf16, name="wkv_sb")

                prev_post = None
                for j in range(NCH):
                    sl = slice(j * CH, (j + 1) * CH)
                    # alloc order matters: the "ps" tag is an 8-deep FIFO, so put
                    # the post()-freed banks (qa, kp) first — the next chunk's cps
                    # then 8-back onto early-freeing banks instead
                    qa_ps = [ps_tile(f"qa_ps{p}") for p in range(2)]
                    kp_ps = ps_tile("kp_ps")
                    cps = [ps_tile(f"cps{ci}") for ci in range(4)]
                    for kg in range(KT // 2):
                        ht1 = p1.tile([128, 2, CH], bf16, tag="ht1", bufs=8, name="ht1")
                        nc.sync.dma_start(ht1[:], hid_d[kg * 256:(kg + 1) * 256, sl]
                                          .rearrange("(g p) t -> p g t", p=128))
                        if j == 0:
                            for k0 in (2 * kg, 2 * kg + 1):
                                nc.sync.dma_start(wkv_sb[:, k0, :],
                                                  wkv_d[k0 * 128:(k0 + 1) * 128, :])
                        if j == 0:
                            nc.scalar.dma_start(
                                wqp_sb[:, kg * 2:(kg + 1) * 2, :],
                                wqp_d[kg * 256:(kg + 1) * 256, :]
                                .rearrange("(k p) n -> p k n", p=128))
                            if kg == 0:
                                # small constants, issued behind the first data tiles
                                nc.scalar.dma_start(ident[:], ident_d)
                                nc.scalar.dma_start(onec[:], onec_d)
                                nc.scalar.dma_start(oner[:], oner_d)
                                nc.scalar.dma_start(ln_sb[:], ln_d)
                        # pass-2 weights trickle in sliced, off the critical path
                        if j == 1:
                            nc.scalar.dma_start(
                                wqn8_sb[:, kg, :, :],
                                wqn8_d[kg * 256:(kg + 1) * 256, :]
                                .rearrange("(i p) n -> p i n", p=128))
                        if j == 2 and kg < 4:
                            nc.scalar.dma_start(masks[:, kg, :], mask_d[:, kg, :])
                        # prefetch pass-2 chunk-0 streams during the last chunk
                        if j == 3:
                            nc.gpsimd.dma_start(ht8_p[:, kg], hid8_d[kg * 256:(kg + 1) * 256, 0:CH]
                                                .rearrange("(i p) t -> p i t", p=128))
                        for ki in range(2):
                            k = 2 * kg + ki
                            st_, sp_ = (k == 0), (k == KT - 1)
                            for ci in range(4):
                                nc.tensor.matmul(cps[ci][:], wkv_sb[:, k, ci * 128:(ci + 1) * 128],
                                                 ht1[:, ki, :], start=st_, stop=sp_)
                            nc.tensor.matmul(kp_ps[:], wkv_sb[:, k, KVR:KVR + 128],
                                             ht1[:, ki, :], start=st_, stop=sp_)
                            for p in range(2):
                                nc.tensor.matmul(qa_ps[p][:], wqp_sb[:, k, p * 128:(p + 1) * 128],
                                                 ht1[:, ki, :], start=st_, stop=sp_)

                    # evacuate raw ckv^T to SBUF promptly (frees the 4 cps banks so
                    # the next chunk's k-loop can start); defer the rest of this
                    # chunk's post-processing behind that k-loop.
                    cos1 = cos_r[j % 2]
                    sin1 = sin_r[j % 2]
                    nc.scalar.dma_start(cos1[:], cos_d[:, sl])
                    nc.scalar.dma_start(sin1[:], sin_d[:, sl])

                    c_sb = []
                    for ci in range(4):
                        t = c_r[(j * 4 + ci) % 8]
                        if ci < 2:
                            nc.scalar.copy(t[:], cps[ci][:])
                        else:
                            nc.vector.tensor_copy(t[:], cps[ci][:])
                        c_sb.append(t)
                    # squares computed eagerly so post()'s var matmul is not
                    # blocked on DVE at the next chunk boundary
                    sqt_l = []
                    for ci in range(4):
                        sqt = p1.tile([128, CH], f32r, tag="sqt", bufs=8, name="sqt")
                        if ci % 2 == 0:
                            nc.vector.tensor_mul(sqt[:], c_sb[ci][:], c_sb[ci][:])
                        else:
                            nc.scalar.square(sqt[:], c_sb[ci][:])
                        sqt_l.append(sqt)

                    def make_post(j, sl, c_sb, sqt_l, kp_ps, qa_ps, cos1, sin1):
                        def post():
                            # q_pe rope (moved from pass 2): rotate-half via
                            # sign-flipped cross-partition copies, then combine
                            for p in range(2):
                                qb_sb = p1.tile([128, CH], f32, tag="qb_sb", bufs=2,
                                                name="qb_sb")
                                for base in (0, 64):
                                    nc.vector.tensor_scalar_mul(
                                        qb_sb[base:base + 32, :],
                                        qa_ps[p][base + 32:base + 64, :], -1.0)
                                    nc.vector.tensor_scalar_mul(
                                        qb_sb[base + 32:base + 64, :],
                                        qa_ps[p][base:base + 32, :], 1.0)
                                qr_t = p1.tile([128, CH], f32, tag="qr_t", bufs=2,
                                               name="qr_t")
                                nc.vector.tensor_mul(qr_t[:], qa_ps[p][:], cos1[:])
                                nc.vector.tensor_mul(qb_sb[:], qb_sb[:], sin1[:])
                                nc.vector.tensor_add(qpr_sb[p][:, sl], qr_t[:], qb_sb[:])
                            # RMSNorm over c (partition direction) via ones-matmul
                            var_ps = ps_tile("var_ps")
                            for ci in range(4):
                                sqt = sqt_l[ci]
                                nc.tensor.matmul(var_ps[0:1, :], r(onec[:]), r(sqt[:]),
                                                 start=(ci == 0), stop=(ci == 3))
                            sd1 = p1.tile([1, CH], f32, tag="sd1", bufs=2, name="sd1")
                            nc.scalar.activation(sd1[:], var_ps[0:1, :], Sqrt, bias=epsb[:],
                                                 scale=1.0 / KVR)
                            iv1 = p1.tile([1, CH], f32r, tag="iv1", bufs=2, name="iv1")
                            nc.vector.reciprocal(iv1[:], sd1[:])
                            bc_sb = p1.tile([128, CH], f32r, tag="bc_sb", bufs=2,
                                            name="bc_sb")
                            nc.gpsimd.partition_broadcast(bc_sb[:], iv1[:])
                            for ci in range(4):
                                nc.vector.scalar_tensor_tensor(
                                    ckvT8[:, ci // 2, ci % 2, sl], c_sb[ci][:],
                                    ln_sb[:, ci:ci + 1], bc_sb[:], op0=mult, op1=mult)
                            # k_pe rope: kper = ka*cos + kb*sin (rows 0:64), then duplicate
                            kr_t = p1.tile([128, CH], f32, tag="kr_t", bufs=2, name="kr_t")
                            nc.vector.tensor_mul(kper[0:64, sl], kp_ps[0:64, :], cos1[0:64, :])
                            nc.vector.tensor_mul(kr_t[0:64, :], kp_ps[64:128, :], sin1[0:64, :])
                            nc.vector.tensor_add(kper[0:64, sl], kper[0:64, sl], kr_t[0:64, :])
                            nc.gpsimd.dma_start(kper[64:128, sl], kper[0:64, sl])

                            # transpose raw ckv^T -> natural [t, c] (one PSUM bank per
                            # t-tile), recompute the variance natural-side via a Square
                            # activation's accumulator, normalize with one DVE op.
                            # (ln is folded into v_b host-side)  PE transposes are
                            # batched first so PE frees before the ACT/DVE tail.
                            tn_list = []
                            for ss in range(4):
                                tn_ps = ps_tile("tn_ps")
                                for ci in range(4):
                                    nc.tensor.transpose(r(tn_ps[:, ci * 128:(ci + 1) * 128]),
                                                        c_sb[ci][:, ss * 128:(ss + 1) * 128],
                                                        ident[:])
                                tn_list.append(tn_ps)
                            for ss in range(4):
                                t_i = 4 * j + ss
                                tn_ps = tn_list[ss]
                                # resident scratch: these are the last-released
                                # tiles of the final post; keeping them out of p1
                                # lets the pass-2 pool allocate earlier
                                sqs = sqs_r[ss % 2]
                                varN = varN_r[ss % 2]
                                nc.scalar.activation(sqs[:], tn_ps[:],
                                                     mybir.ActivationFunctionType.Square,
                                                     accum_out=varN[:])
                                sdN = sdN_r[ss % 2]
                                nc.scalar.activation(sdN[:], varN[:], Sqrt, bias=eps128[:],
                                                     scale=1.0 / KVR)
                                ivN = ivN_r[ss % 2]
                                nc.vector.reciprocal(ivN[:], sdN[:])
                                nc.vector.tensor_scalar_mul(ckvN[t_i][:], tn_ps[:], ivN[:])
                        return post

                    post_j = make_post(j, sl, c_sb, sqt_l, kp_ps, qa_ps, cos1, sin1)
                    if prev_post is not None:
                        prev_post()
                    prev_post = post_j

                # chunk-0 q_nope projection (resident/PSUM inputs only) issues
                # ahead of the final chunk's post so PE overlaps its DVE chain
                qn_ps_cur = [ps_tile(f"qn_ps{h}") for h in range(HL)]
                for kg in range(KT // 2):
                    for h in range(HL):
                        nc.tensor.matmul(qn_ps_cur[h][:],
                                         wqn8_sb[:, kg, :, h * 128:(h + 1) * 128],
                                         ht8_p[:, kg], start=(kg == 0),
                                         stop=(kg == KT // 2 - 1), perf_mode=DR2)
                prev_post()

              nc.scalar.dma_start(kb_sb[:], kb_d.rearrange("h d c -> d h c"))
              nc.scalar.dma_start(vb_sb[:], vb_d.rearrange("h (ci p) d -> p h ci d", p=128))

              # ---------------- pass 2: q proj + attention + o_proj -----------
              with tc.tile_pool(name="p2", bufs=1) as p2:
                for j in range(NCH):
                    sl = slice(j * CH, (j + 1) * CH)

                    # evacuate q_nope (projected during the previous chunk, or in
                    # pass 1's tail for chunk 0)
                    qn_sb = []
                    for h in range(HL):
                        t = p2.tile([128, CH], f32r, tag="qn_sb", bufs=4, name=f"qn_sb{h}")
                        nc.scalar.copy(t[:], qn_ps_cur[h][:])
                        qn_sb.append(t)

                    # issue next chunk's fp8 hidden stream with a full chunk of lead
                    if j + 1 < NCH:
                        ht8c = p2.tile([128, KT // 2, 2, CH], f8, tag="ht8c", bufs=2,
                                       name="ht8c")
                        for kg in range(KT // 2):
                            nc.sync.dma_start(
                                ht8c[:, kg],
                                hid8_d[kg * 256:(kg + 1) * 256, (j + 1) * CH:(j + 2) * CH]
                                .rearrange("(i p) t -> p i t", p=128))

                    vo_sb = p2.tile([128, HL, CH], bf16, tag="vo_sb", bufs=1, name="vo_sb")
                    prev_tail = None
                    for h in range(HL):
                        # q_lat^T[c, s] per head, quantized to fp8 DoubleRow layout
                        ql8 = p2.tile([128, 2, 2, CH], f8, tag="ql8", bufs=2, name="ql8")
                        for ci in range(4):
                            ql_ps = ps_tile("ql_ps")
                            nc.tensor.matmul(ql_ps[:], r(kb_sb[:, h, ci * 128:(ci + 1) * 128]),
                                             r(qn_sb[h][:]), start=True, stop=True)
                            if ci % 2 == 0:
                                nc.scalar.copy(ql8[:, ci // 2, ci % 2, :], ql_ps[:])
                            else:
                                nc.vector.tensor_copy(ql8[:, ci // 2, ci % 2, :], ql_ps[:])

                        hp, hh = h // 2, (h % 2) * 64
                        ol_ps = []
                        rs_box = []
                        # t-tile order: diagonal tiles first (first is full-width,
                        # carries start=True), then the off-diagonal history tiles.
                        tts = list(range(4 * j, 4 * j + 4)) + list(range(0, 4 * j))

                        def score_exp(idx):
                            t_i = tts[idx]
                            kd = t_i - 4 * j
                            st = 0 if kd < 0 else (0, 128, 256, 256)[kd]
                            sc_ps = ps_tile("sc_ps")
                            for c2 in range(2):
                                nc.tensor.matmul(sc_ps[:, st:],
                                                 ckvT8[:, c2, :, t_i * 128:(t_i + 1) * 128],
                                                 ql8[:, c2, :, st:], start=(c2 == 0),
                                                 stop=False, perf_mode=DR2)
                            nc.tensor.matmul(sc_ps[:, st:],
                                             kper[hh:hh + 64, t_i * 128:(t_i + 1) * 128],
                                             qpr_sb[hp][hh:hh + 64, j * CH + st:(j + 1) * CH],
                                             start=False, stop=True)
                            if kd >= 0:
                                nc.vector.tensor_add(sc_ps[:, st:], sc_ps[:, st:], masks[:, kd, st:])
                            ex_sb = p2.tile([128, CH], f32r, tag="ex_sb", bufs=4, name="ex_sb")
                            nc.scalar.activation(ex_sb[:, st:], sc_ps[:, st:], Exp,
                                                 bias=zb128[:], scale=SCALE)
                            return ex_sb, st

                        def pv(idx, ex_sb, st):
                            t_i = tts[idx]
                            first, last = (idx == 0), (idx == len(tts) - 1)
                            for ci in range(4):
                                nc.tensor.matmul(ol_ps[ci][:, st:], r(ckvN[t_i][:, ci * 128:(ci + 1) * 128]),
                                                 r(ex_sb[:, st:]), start=first, stop=last)
                            nc.tensor.matmul(rs_box[0][0:1, st:], r(onec[:]), r(ex_sb[:, st:]),
                                             start=first, stop=last)

                        # overlap previous head's tail behind this head's q_lat and
                        # first two score tiles (ol/rs banks allocate only after the
                        # previous head's are released inside prev_tail)
                        npre = min(3, len(tts))
                        pends = [(i,) + score_exp(i) for i in range(npre)]
                        if prev_tail is not None:
                            prev_tail()
                            prev_tail = None
                        ol_ps.extend(ps_tile(f"ol_ps{ci}") for ci in range(4))
                        rs_box.append(ps_tile("rs_ps"))
                        for idx in range(npre, len(tts)):
                            pends.append((idx,) + score_exp(idx))
                            if len(pends) > 5:
                                pv(*pends.pop(0))
                        for pend in pends:
                            pv(*pend)

                        def make_tail(h, ol_ps, rs_ps):
                            def tail():
                                # evacuate unnormalized out_lat (starts right after last PV)
                                ol_sb = p2.tile([128, 4, CH], f32r, tag="ol_sb", bufs=1, name="ol_sb")
                                for ci in range(4):
                                    if ci % 2 == 0:
                                        nc.scalar.copy(ol_sb[:, ci, :], ol_ps[ci][:])
                                    else:
                                        nc.vector.tensor_copy(ol_sb[:, ci, :], ol_ps[ci][:])
                                # softmax denominator -> broadcast tile (parallel chain)
                                rv_sb = p2.tile([1, CH], f32r, tag="rv_sb", bufs=1, name="rv_sb")
                                nc.vector.reciprocal(rv_sb[:], rs_ps[0:1, :])
                                bc2_sb = p2.tile([128, CH], f32r, tag="bc2_sb", bufs=1, name="bc2_sb")
                                nc.gpsimd.partition_broadcast(bc2_sb[:], rv_sb[:])
                                # v_b expansion on unnormalized out_lat; normalize once on
                                # v_out (per-column scaling commutes with the contraction)
                                vo_ps = ps_tile("vo_ps")
                                for ci in range(4):
                                    nc.tensor.matmul(vo_ps[:], r(vb_sb[:, h, ci, :]), r(ol_sb[:, ci, :]),
                                                     start=(ci == 0), stop=(ci == 3))
                                nc.vector.tensor_mul(vo_sb[:, h, :], vo_ps[:], bc2_sb[:])
                            return tail

                        prev_tail = make_tail(h, ol_ps, rs_box[0])
                    prev_tail()

                    # project next chunk's q_nope before o_proj so its PSUM banks
                    # allocate ahead of oo_ps in the bank FIFO
                    if j + 1 < NCH:
                        qn_ps_cur = [ps_tile(f"qn_ps{h}") for h in range(HL)]
                        for kg in range(KT // 2):
                            for h in range(HL):
                                nc.tensor.matmul(qn_ps_cur[h][:],
                                                 wqn8_sb[:, kg, :, h * 128:(h + 1) * 128],
                                                 ht8c[:, kg], start=(kg == 0),
                                                 stop=(kg == KT // 2 - 1), perf_mode=DR2)

                    # o_proj partial: out^T[hid, s] = sum_h wo^T.T @ v_out^T
                    for htile in range(KT):
                        wo_sb = p2.tile([128, HL, 128], bf16, tag="wo_sb", bufs=5, name="wo_sb")
                        nc.sync.dma_start(wo_sb[:], wo_d[:, htile * 128:(htile + 1) * 128]
                                          .rearrange("(a p) n -> p a n", p=128))
                        oo_ps = ps_tile("oo_ps")
                        for hh2 in range(HL):
                            nc.tensor.matmul(oo_ps[:], wo_sb[:, hh2, :], vo_sb[:, hh2, :],
                                             start=(hh2 == 0), stop=(hh2 == HL - 1))
                        oo_sb = p2.tile([128, CH], bf16, tag="oo_sb", bufs=3, name="oo_sb")
                        nc.vector.tensor_copy(oo_sb[:], oo_ps[:])
                        nc.scalar.dma_start(out_d[htile * 128:(htile + 1) * 128, sl], oo_sb[:])

    nc.compile()
    return nc


# ---------------------------------------------------------------------------
# host-side input prep / output assembly
# ---------------------------------------------------------------------------
_PERM = np.concatenate([np.arange(0, DR, 2), np.arange(1, DR, 2)])


def _rope_tables(pos, s):
    inv_freq = 1.0 / (THETA ** (np.arange(0, DR, 2, dtype=np.float64) / DR))
    t = pos.astype(np.float64)
    freqs = t[:, None] * inv_freq
    emb = np.concatenate([freqs, freqs], axis=-1)          # [s, DR]
    cosT = np.cos(emb).T.astype(np.float32)                # [DR, s]
    sinT = np.sin(emb).T.astype(np.float32)
    cos_p = np.ascontiguousarray(np.vstack([cosT, cosT]))  # [128, s]
    sin_p = np.ascontiguousarray(np.vstack([sinT, sinT]))
    return cos_p, sin_p


def _masks():
    t = np.arange(128)[:, None]
    c = np.arange(CH)[None, :]
    m = np.zeros((128, 4, CH), np.float32)
    for kd in range(4):
        m[:, kd, :] = np.where(c >= 128 * kd + t, 0.0, -1e30).astype(np.float32)
    return m


def prep_core_inputs(inputs, core, s=S, hid=HID):
    import ml_dtypes

    b, g = core // 4, core % 4
    heads = slice(HL * g, HL * (g + 1))
    hs = np.asarray(inputs["hidden_states"], np.float32)[b, :s, :hid]
    hid_t = np.ascontiguousarray(hs.T)
    m = {"hid_t": hid_t.astype(ml_dtypes.bfloat16),
         "hid8_t": hid_t.astype(ml_dtypes.float8_e4m3)}

    # scale 0.02-std weights out of e4m3's subnormal range; 1/64 folds into k_b
    wq = np.asarray(inputs["q_nope_weight"], np.float32).reshape(H, DN, HID)[heads, :, :hid]
    wq_t = wq.transpose(2, 0, 1).reshape(hid, HL * DN)
    m["wqn8_t"] = np.ascontiguousarray(wq_t * 64.0).astype(ml_dtypes.float8_e4m3)

    wqp = np.asarray(inputs["q_pe_weight"], np.float32).reshape(H, DR, HID)[heads, :, :hid]
    a = wqp[:, _PERM, :]                                   # [4, 64, hid]
    bv = np.concatenate([-a[:, 32:64], a[:, 0:32]], axis=1)
    A = a.reshape(2, 128, hid)
    Bv = bv.reshape(2, 128, hid)
    wqpe_t = np.concatenate([A[0], A[1]], axis=0).T
    m["wqp_t"] = np.ascontiguousarray(wqpe_t).astype(ml_dtypes.bfloat16)

    wkv = np.asarray(inputs["kv_a_weight"], np.float32)[:, :hid]
    kpe_a = wkv[KVR:][_PERM]
    kpe_b = np.concatenate([-kpe_a[32:], kpe_a[:32]], axis=0)
    m["wkv_t"] = np.ascontiguousarray(
        np.concatenate([wkv[:KVR], kpe_a, kpe_b], axis=0).T).astype(ml_dtypes.bfloat16)

    m["ln_t"] = np.ascontiguousarray(
        np.asarray(inputs["kv_a_ln_weight"], np.float32).reshape(4, 128).T)
    m["kb"] = np.ascontiguousarray(
        np.asarray(inputs["k_b_weight"], np.float32)[heads] / 64.0)
    # ckvN on-device omits the ln weight; fold it into v_b's KVR axis instead
    ln_w = np.asarray(inputs["kv_a_ln_weight"], np.float32)
    m["vb_t"] = np.ascontiguousarray(
        np.asarray(inputs["v_b_weight"], np.float32)[heads].transpose(0, 2, 1)
        * ln_w[None, :, None])
    m["wo_t"] = np.ascontiguousarray(
        np.asarray(inputs["o_weight"], np.float32)[:hid, HL * DV * g:HL * DV * (g + 1)].T
    ).astype(ml_dtypes.bfloat16)

    pos = np.asarray(inputs["position_ids"]).reshape(-1)[:s]
    cos_p, sin_p = _rope_tables(pos, s)
    m["cos_p"], m["sin_p"] = cos_p, sin_p
    m["masks"] = _masks()
    m["ident"] = np.eye(128, dtype=np.float32)
    m["ones_c"] = np.ones((128, 1), np.float32)
    m["ones_r"] = np.ones((1, 128), np.float32)
    return m


_NC_CACHE = {}


def _get_nc():
    if "nc" not in _NC_CACHE:
        _NC_CACHE["nc"] = build_nc()
    return _NC_CACHE["nc"]


def kernel(**inputs):
    from concourse import bass_utils

    nc = _get_nc()
    in_maps = [prep_core_inputs(inputs, c) for c in range(NCORES)]
    res = bass_utils.run_bass_kernel_spmd(nc, in_maps, core_ids=list(range(NCORES)))
    out = np.empty((B, S, HID), np.float32)
    for b in range(B):
        acc = np.array(res.results[4 * b]["out_t"], np.float32)
        for g in range(1, 4):
            acc += res.results[4 * b + g]["out_t"]
        out[b] = acc.T
    return out



# revision 115
# speedup vs baseline: 1.0201x; 1.0003x over previous
"""DeepseekV2 MLA attention (weight-absorbed, MQA-style latent) on 8 TRN2 NeuronCores.

Sharding: data-parallel over batch (B=2) x tensor-parallel over heads (4 heads/core).
Each core computes, for its batch element and its 4 heads, the partial o_proj
output out_t = [HID, S] (transposed layout); the host sums the 4 partials per
batch element and transposes back.

Precision plan (tolerance 2e-2; measured ~9.8e-3):
  The values path (ckv -> PV -> v_b) stays fp32r, except the hidden/wkv
  projection stream and the v_out/o_proj weights which are bf16 (~0.3% value
  noise).  The attention-score path is fp8 e4m3 with DoubleRow perf mode
  (0.5 cycles/row, 256-deep contraction = 4x fp32r matmul throughput):
  q_nope projection from fp8 hidden/weights (weights pre-scaled x64 out of
  e4m3's subnormal range, compensated in k_b), and q_lat x ckv score
  contractions.  RoPE q/k score operands are bf16.  exp stays fp32.

Structure (per core):
  pass 1 streams bf16 hidden^T once: ckv^T (fp32 psum), k_pe rope, and the
  rope-q projection+rotation (resident bf16 qpr/kper).  Per s-chunk post:
  row-layout RMS stats via ones-matmul + Pool-engine partition_broadcast, fp8
  normed keys ckvT8 [c,(2,2),S]; PE transposes into one PSUM bank per t-tile,
  natural-layout variance via a Square activation's accumulator, one DVE op
  normalizes ckvN [t,c] (ln is folded into v_b host-side).  The chunk-0
  q_nope fp8 DoubleRow projection is hoisted into pass 1's tail (prefetched
  fp8 hidden, resident fp8 weights) to overlap the final post's DVE chain.
  pass 2 per chunk: evacuate q_nope, per head: q_lat (fp32r) quantized into
  fp8 DoubleRow layout, scores = 2 fp8 DoubleRow + 1 bf16 rope matmul per
  128-key tile, max-free exp (magnitudes ~5 std, verified on host), PV +
  ones-matmul rowsum in fp32r, v_b expansion on unnormalized out_lat with a
  single post-normalization (partition_broadcast of the reciprocal rowsum).
  The next chunk's q_nope projection runs between the head loop and o_proj
  so its PSUM banks allocate ahead of oo_ps in the 8-bank FIFO.
"""
import sys

for _p in ("/opt/trn_rl_repo", "/root/.axon_site/_ro/trn_rl_repo"):
    if _p not in sys.path:
        sys.path.insert(0, _p)

import numpy as np

B, S, HID = 2, 2048, 2048
H, DN, DR, KVR, DV = 16, 128, 64, 512, 128
THETA, EPS = 10000.0, 1e-6
SCALE = float((DN + DR) ** -0.5)
NCORES, HL = 8, 4  # 2 (batch) x 4 (head groups of 4)
CH = 512           # s-chunk width (= max fp32 moving operand)


def build_nc(s=S, hid=HID, reps=1):
    import concourse.bacc as bacc
    import concourse.mybir as mybir
    from concourse import tile

    f32 = mybir.dt.float32
    f32r = mybir.dt.float32r
    f8 = mybir.dt.float8e4
    bf16 = mybir.dt.bfloat16
    DR2 = mybir.MatmulPerfMode.DoubleRow
    Exp = mybir.ActivationFunctionType.Exp
    Sqrt = mybir.ActivationFunctionType.Sqrt
    mult = mybir.AluOpType.mult

    def r(ap):
        return ap.bitcast(f32r)

    NCH = s // CH      # s-chunks
    KT = hid // 128    # contraction tiles over HID
    NT = s // 128      # t-tiles

    nc = bacc.Bacc("TRN2", target_bir_lowering=False, debug=False,
                   enable_asserts=False, num_devices=NCORES)

    hid_d = nc.dram_tensor("hid_t", [hid, s], bf16, kind="ExternalInput").ap()
    hid8_d = nc.dram_tensor("hid8_t", [hid, s], f8, kind="ExternalInput").ap()
    wqn8_d = nc.dram_tensor("wqn8_t", [hid, HL * DN], f8, kind="ExternalInput").ap()
    wqp_d = nc.dram_tensor("wqp_t", [hid, 256], bf16, kind="ExternalInput").ap()
    wkv_d = nc.dram_tensor("wkv_t", [hid, KVR + 2 * DR], bf16, kind="ExternalInput").ap()
    ln_d = nc.dram_tensor("ln_t", [128, 4], f32, kind="ExternalInput").ap()
    kb_d = nc.dram_tensor("kb", [HL, DN, KVR], f32r, kind="ExternalInput").ap()
    vb_d = nc.dram_tensor("vb_t", [HL, KVR, DV], f32r, kind="ExternalInput").ap()
    wo_d = nc.dram_tensor("wo_t", [HL * DV, hid], bf16, kind="ExternalInput").ap()
    cos_d = nc.dram_tensor("cos_p", [128, s], f32, kind="ExternalInput").ap()
    sin_d = nc.dram_tensor("sin_p", [128, s], f32, kind="ExternalInput").ap()
    mask_d = nc.dram_tensor("masks", [128, 4, CH], f32, kind="ExternalInput").ap()
    ident_d = nc.dram_tensor("ident", [128, 128], f32r, kind="ExternalInput").ap()
    onec_d = nc.dram_tensor("ones_c", [128, 1], f32r, kind="ExternalInput").ap()
    oner_d = nc.dram_tensor("ones_r", [1, 128], f32r, kind="ExternalInput").ap()
    out_d = nc.dram_tensor("out_t", [hid, s], bf16, kind="ExternalOutput").ap()

    with tile.TileContext(nc) as tc, \
         nc.allow_low_precision(reason="f32r-typed tiles feed fp32r matmuls; psum accum stays fp32"):
        with tc.tile_pool(name="res", bufs=1) as res, \
             tc.tile_pool(name="psp", bufs=8, space="PSUM") as psp:

            def ps_tile(name):
                return psp.tile([128, CH], f32, tag="ps", name=name)

            # resident tiles
            ckvT8 = res.tile([128, 2, 2, s], f8, name="ckvT8")
            wqn8_sb = res.tile([128, KT // 2, 2, HL * DN], f8, name="wqn8_sb")
            wqp_sb = res.tile([128, KT, 256], bf16, name="wqp_sb")
            kper = res.tile([128, s], bf16, name="kper")
            qpr_sb = [res.tile([128, s], bf16, name=f"qpr_sb{p}") for p in range(2)]
            ckvN = [res.tile([128, KVR], f32r, name=f"ckvN{t}") for t in range(NT)]
            kb_sb = res.tile([128, HL, KVR], f32r, name="kb_sb")
            vb_sb = res.tile([128, HL, 4, DV], f32r, name="vb_sb")
            ident = res.tile([128, 128], f32r, name="ident_sb")
            onec = res.tile([128, 1], f32r, name="onec_sb")
            oner = res.tile([1, 128], f32r, name="oner_sb")
            ln_sb = res.tile([128, 4], f32, name="ln_sb")
            ht8_p = res.tile([128, KT // 2, 2, CH], f8, name="ht8_p")  # pass-2 chunk-0 prefetch
            masks = res.tile([128, 4, CH], f32, name="masks_sb")
            sqs_r = [res.tile([128, CH], f32, name=f"sqs_r{i}") for i in range(2)]
            c_r = [res.tile([128, CH], f32r, name=f"c_r{i}") for i in range(8)]
            cos_r = [res.tile([128, CH], f32, name=f"cos_r{i}") for i in range(2)]
            sin_r = [res.tile([128, CH], f32, name=f"sin_r{i}") for i in range(2)]
            varN_r = [res.tile([128, 1], f32, name=f"varN_r{i}") for i in range(2)]
            sdN_r = [res.tile([128, 1], f32, name=f"sdN_r{i}") for i in range(2)]
            ivN_r = [res.tile([128, 1], f32, name=f"ivN_r{i}") for i in range(2)]
            zb128 = res.tile([128, 1], f32, name="zb128")
            epsb = res.tile([1, 1], f32, name="epsb")
            eps128 = res.tile([128, 1], f32, name="eps128")
            nc.vector.memset(zb128[:], 0.0)
            nc.vector.memset(epsb[:], EPS)
            nc.vector.memset(eps128[:], EPS)

            # ---------------- pass 1: latent KV (ckv^T, ckv_nat, k_pe rot) ----
            for _rep in range(reps):
              with tc.tile_pool(name="p1", bufs=1) as p1:
                wkv_sb = p1.tile([128, KT, KVR + 2 * DR], bf16, name="wkv_sb")

                prev_post = None
                for j in range(NCH):
                    sl = slice(j * CH, (j + 1) * CH)
                    # alloc order matters: the "ps" tag is an 8-deep FIFO, so put
                    # the post()-freed banks (qa, kp) first — the next chunk's cps
                    # then 8-back onto early-freeing banks instead
                    qa_ps = [ps_tile(f"qa_ps{p}") for p in range(2)]
                    kp_ps = ps_tile("kp_ps")
                    cps = [ps_tile(f"cps{ci}") for ci in range(4)]
                    for kg in range(KT // 2):
                        ht1 = p1.tile([128, 2, CH], bf16, tag="ht1", bufs=8, name="ht1")
                        nc.sync.dma_start(ht1[:], hid_d[kg * 256:(kg + 1) * 256, sl]
                                          .rearrange("(g p) t -> p g t", p=128))
                        if j == 0:
                            for k0 in (2 * kg, 2 * kg + 1):
                                nc.sync.dma_start(wkv_sb[:, k0, :],
                                                  wkv_d[k0 * 128:(k0 + 1) * 128, :])
                        if j == 0:
                            nc.scalar.dma_start(
                                wqp_sb[:, kg * 2:(kg + 1) * 2, :],
                                wqp_d[kg * 256:(kg + 1) * 256, :]
                                .rearrange("(k p) n -> p k n", p=128))
                            if kg == 0:
                                # small constants, issued behind the first data tiles
                                nc.scalar.dma_start(ident[:], ident_d)
                                nc.scalar.dma_start(onec[:], onec_d)
                                nc.scalar.dma_start(oner[:], oner_d)
                                nc.scalar.dma_start(ln_sb[:], ln_d)
                        # pass-2 weights trickle in sliced, off the critical path
                        if j == 1:
                            nc.scalar.dma_start(
                                wqn8_sb[:, kg, :, :],
                                wqn8_d[kg * 256:(kg + 1) * 256, :]
                                .rearrange("(i p) n -> p i n", p=128))
                        if j == 2 and kg < 4:
                            nc.scalar.dma_start(masks[:, kg, :], mask_d[:, kg, :])
                        # prefetch pass-2 chunk-0 streams during the last chunk
                        if j == 3:
                            nc.gpsimd.dma_start(ht8_p[:, kg], hid8_d[kg * 256:(kg + 1) * 256, 0:CH]
                                                .rearrange("(i p) t -> p i t", p=128))
                        for ki in range(2):
                            k = 2 * kg + ki
                            st_, sp_ = (k == 0), (k == KT - 1)
                            for ci in range(4):
                                nc.tensor.matmul(cps[ci][:], wkv_sb[:, k, ci * 128:(ci + 1) * 128],
                                                 ht1[:, ki, :], start=st_, stop=sp_)
                            nc.tensor.matmul(kp_ps[:], wkv_sb[:, k, KVR:KVR + 128],
                                             ht1[:, ki, :], start=st_, stop=sp_)
                            for p in range(2):
                                nc.tensor.matmul(qa_ps[p][:], wqp_sb[:, k, p * 128:(p + 1) * 128],
                                                 ht1[:, ki, :], start=st_, stop=sp_)

                    # evacuate raw ckv^T to SBUF promptly (frees the 4 cps banks so
                    # the next chunk's k-loop can start); defer the rest of this
                    # chunk's post-processing behind that k-loop.
                    cos1 = cos_r[j % 2]
                    sin1 = sin_r[j % 2]
                    nc.scalar.dma_start(cos1[:], cos_d[:, sl])
                    nc.scalar.dma_start(sin1[:], sin_d[:, sl])

                    c_sb = []
                    for ci in range(4):
                        t = c_r[(j * 4 + ci) % 8]
                        if ci < 2:
                            nc.scalar.copy(t[:], cps[ci][:])
                        else:
                            nc.vector.tensor_copy(t[:], cps[ci][:])
                        c_sb.append(t)
                    # squares computed eagerly so post()'s var matmul is not
                    # blocked on DVE at the next chunk boundary
                    sqt_l = []
                    for ci in range(4):
                        sqt = p1.tile([128, CH], f32r, tag="sqt", bufs=8, name="sqt")
                        if ci % 2 == 0:
                            nc.vector.tensor_mul(sqt[:], c_sb[ci][:], c_sb[ci][:])
                        else:
                            nc.scalar.square(sqt[:], c_sb[ci][:])
                        sqt_l.append(sqt)

                    def make_post(j, sl, c_sb, sqt_l, kp_ps, qa_ps, cos1, sin1):
                        def post():
                            # q_pe rope (moved from pass 2): rotate-half via
                            # sign-flipped cross-partition copies, then combine
                            for p in range(2):
                                qb_sb = p1.tile([128, CH], f32, tag="qb_sb", bufs=2,
                                                name="qb_sb")
                                for base in (0, 64):
                                    nc.vector.tensor_scalar_mul(
                                        qb_sb[base:base + 32, :],
                                        qa_ps[p][base + 32:base + 64, :], -1.0)
                                    nc.vector.tensor_scalar_mul(
                                        qb_sb[base + 32:base + 64, :],
                                        qa_ps[p][base:base + 32, :], 1.0)
                                qr_t = p1.tile([128, CH], f32, tag="qr_t", bufs=2,
                                               name="qr_t")
                                nc.vector.tensor_mul(qr_t[:], qa_ps[p][:], cos1[:])
                                nc.vector.tensor_mul(qb_sb[:], qb_sb[:], sin1[:])
                                nc.vector.tensor_add(qpr_sb[p][:, sl], qr_t[:], qb_sb[:])
                            # RMSNorm over c (partition direction) via ones-matmul
                            var_ps = ps_tile("var_ps")
                            for ci in range(4):
                                sqt = sqt_l[ci]
                                nc.tensor.matmul(var_ps[0:1, :], r(onec[:]), r(sqt[:]),
                                                 start=(ci == 0), stop=(ci == 3))
                            sd1 = p1.tile([1, CH], f32, tag="sd1", bufs=2, name="sd1")
                            nc.scalar.activation(sd1[:], var_ps[0:1, :], Sqrt, bias=epsb[:],
                                                 scale=1.0 / KVR)
                            iv1 = p1.tile([1, CH], f32r, tag="iv1", bufs=2, name="iv1")
                            nc.vector.reciprocal(iv1[:], sd1[:])
                            bc_sb = p1.tile([128, CH], f32r, tag="bc_sb", bufs=2,
                                            name="bc_sb")
                            nc.gpsimd.partition_broadcast(bc_sb[:], iv1[:])
                            for ci in range(4):
                                nc.vector.scalar_tensor_tensor(
                                    ckvT8[:, ci // 2, ci % 2, sl], c_sb[ci][:],
                                    ln_sb[:, ci:ci + 1], bc_sb[:], op0=mult, op1=mult)
                            # k_pe rope: kper = ka*cos + kb*sin (rows 0:64), then duplicate
                            kr_t = p1.tile([128, CH], f32, tag="kr_t", bufs=2, name="kr_t")
                            nc.vector.tensor_mul(kper[0:64, sl], kp_ps[0:64, :], cos1[0:64, :])
                            nc.vector.tensor_mul(kr_t[0:64, :], kp_ps[64:128, :], sin1[0:64, :])
                            nc.vector.tensor_add(kper[0:64, sl], kper[0:64, sl], kr_t[0:64, :])
                            nc.gpsimd.dma_start(kper[64:128, sl], kper[0:64, sl])

                            # transpose raw ckv^T -> natural [t, c] (one PSUM bank per
                            # t-tile), recompute the variance natural-side via a Square
                            # activation's accumulator, normalize with one DVE op.
                            # (ln is folded into v_b host-side)  PE transposes are
                            # batched first so PE frees before the ACT/DVE tail.
                            tn_list = []
                            for ss in range(4):
                                tn_ps = ps_tile("tn_ps")
                                for ci in range(4):
                                    nc.tensor.transpose(r(tn_ps[:, ci * 128:(ci + 1) * 128]),
                                                        c_sb[ci][:, ss * 128:(ss + 1) * 128],
                                                        ident[:])
                                tn_list.append(tn_ps)
                            for ss in range(4):
                                t_i = 4 * j + ss
                                tn_ps = tn_list[ss]
                                # resident scratch: these are the last-released
                                # tiles of the final post; keeping them out of p1
                                # lets the pass-2 pool allocate earlier
                                sqs = sqs_r[ss % 2]
                                varN = varN_r[ss % 2]
                                nc.scalar.activation(sqs[:], tn_ps[:],
                                                     mybir.ActivationFunctionType.Square,
                                                     accum_out=varN[:])
                                sdN = sdN_r[ss % 2]
                                nc.scalar.activation(sdN[:], varN[:], Sqrt, bias=eps128[:],
                                                     scale=1.0 / KVR)
                                ivN = ivN_r[ss % 2]
                                nc.vector.reciprocal(ivN[:], sdN[:])
                                nc.vector.tensor_scalar_mul(ckvN[t_i][:], tn_ps[:], ivN[:])
                        return post

                    post_j = make_post(j, sl, c_sb, sqt_l, kp_ps, qa_ps, cos1, sin1)
                    if prev_post is not None:
                        prev_post()
                    prev_post = post_j

                # chunk-0 q_nope projection (resident/PSUM inputs only) issues
                # ahead of the final chunk's post so PE overlaps its DVE chain
                qn_ps_cur = [ps_tile(f"qn_ps{h}") for h in range(HL)]
                for kg in range(KT // 2):
                    for h in range(HL):
                        nc.tensor.matmul(qn_ps_cur[h][:],
                                         wqn8_sb[:, kg, :, h * 128:(h + 1) * 128],
                                         ht8_p[:, kg], start=(kg == 0),
                                         stop=(kg == KT // 2 - 1), perf_mode=DR2)
                prev_post()

              nc.scalar.dma_start(kb_sb[:], kb_d.rearrange("h d c -> d h c"))
              nc.scalar.dma_start(vb_sb[:], vb_d.rearrange("h (ci p) d -> p h ci d", p=128))

              # ---------------- pass 2: q proj + attention + o_proj -----------
              with tc.tile_pool(name="p2", bufs=1) as p2:
                for j in range(NCH):
                    sl = slice(j * CH, (j + 1) * CH)

                    # evacuate q_nope (projected during the previous chunk, or in
                    # pass 1's tail for chunk 0)
                    qn_sb = []
                    for h in range(HL):
                        t = p2.tile([128, CH], f32r, tag="qn_sb", bufs=4, name=f"qn_sb{h}")
                        nc.scalar.copy(t[:], qn_ps_cur[h][:])
                        qn_sb.append(t)

                    # issue next chunk's fp8 hidden stream with a full chunk of lead
                    if j + 1 < NCH:
                        ht8c = p2.tile([128, KT // 2, 2, CH], f8, tag="ht8c", bufs=2,
                                       name="ht8c")
                        for kg in range(KT // 2):
                            nc.sync.dma_start(
                                ht8c[:, kg],
                                hid8_d[kg * 256:(kg + 1) * 256, (j + 1) * CH:(j + 2) * CH]
                                .rearrange("(i p) t -> p i t", p=128))

                    vo_sb = p2.tile([128, HL, CH], bf16, tag="vo_sb", bufs=1, name="vo_sb")
                    prev_tail = None
                    for h in range(HL):
                        # q_lat^T[c, s] per head, quantized to fp8 DoubleRow layout
                        ql8 = p2.tile([128, 2, 2, CH], f8, tag="ql8", bufs=2, name="ql8")
                        for ci in range(4):
                            ql_ps = ps_tile("ql_ps")
                            nc.tensor.matmul(ql_ps[:], r(kb_sb[:, h, ci * 128:(ci + 1) * 128]),
                                             r(qn_sb[h][:]), start=True, stop=True)
                            if ci % 2 == 0:
                                nc.scalar.copy(ql8[:, ci // 2, ci % 2, :], ql_ps[:])
                            else:
                                nc.vector.tensor_copy(ql8[:, ci // 2, ci % 2, :], ql_ps[:])

                        hp, hh = h // 2, (h % 2) * 64
                        ol_ps = []
                        rs_box = []
                        # t-tile order: diagonal tiles first (first is full-width,
                        # carries start=True), then the off-diagonal history tiles.
                        tts = list(range(4 * j, 4 * j + 4)) + list(range(0, 4 * j))

                        def score_exp(idx):
                            t_i = tts[idx]
                            kd = t_i - 4 * j
                            st = 0 if kd < 0 else (0, 128, 256, 256)[kd]
                            sc_ps = ps_tile("sc_ps")
                            for c2 in range(2):
                                nc.tensor.matmul(sc_ps[:, st:],
                                                 ckvT8[:, c2, :, t_i * 128:(t_i + 1) * 128],
                                                 ql8[:, c2, :, st:], start=(c2 == 0),
                                                 stop=False, perf_mode=DR2)
                            nc.tensor.matmul(sc_ps[:, st:],
                                             kper[hh:hh + 64, t_i * 128:(t_i + 1) * 128],
                                             qpr_sb[hp][hh:hh + 64, j * CH + st:(j + 1) * CH],
                                             start=False, stop=True)
                            if kd >= 0:
                                nc.vector.tensor_add(sc_ps[:, st:], sc_ps[:, st:], masks[:, kd, st:])
                            ex_sb = p2.tile([128, CH], f32r, tag="ex_sb", bufs=4, name="ex_sb")
                            nc.scalar.activation(ex_sb[:, st:], sc_ps[:, st:], Exp,
                                                 bias=zb128[:], scale=SCALE)
                            return ex_sb, st

                        def pv(idx, ex_sb, st):
                            t_i = tts[idx]
                            first, last = (idx == 0), (idx == len(tts) - 1)
                            for ci in range(4):
                                nc.tensor.matmul(ol_ps[ci][:, st:], r(ckvN[t_i][:, ci * 128:(ci + 1) * 128]),
                                                 r(ex_sb[:, st:]), start=first, stop=last)
                            nc.tensor.matmul(rs_box[0][0:1, st:], r(onec[:]), r(ex_sb[:, st:]),
                                             start=first, stop=last)

                        # overlap previous head's tail behind this head's q_lat and
                        # first two score tiles (ol/rs banks allocate only after the
                        # previous head's are released inside prev_tail)
                        npre = min(3, len(tts))
                        pends = [(i,) + score_exp(i) for i in range(npre)]
                        if prev_tail is not None:
                            prev_tail()
                            prev_tail = None
                        ol_ps.extend(ps_tile(f"ol_ps{ci}") for ci in range(4))
                        rs_box.append(ps_tile("rs_ps"))
                        for idx in range(npre, len(tts)):
                            pends.append((idx,) + score_exp(idx))
                            if len(pends) > 5:
                                pv(*pends.pop(0))
                        for pend in pends:
                            pv(*pend)

                        def make_tail(h, ol_ps, rs_ps):
                            def tail():
                                # evacuate unnormalized out_lat (starts right after last PV)
                                ol_sb = p2.tile([128, 4, CH], f32r, tag="ol_sb", bufs=1, name="ol_sb")
                                for ci in range(4):
                                    if ci % 2 == 0:
                                        nc.scalar.copy(ol_sb[:, ci, :], ol_ps[ci][:])
                                    else:
                                        nc.vector.tensor_copy(ol_sb[:, ci, :], ol_ps[ci][:])
                                # softmax denominator -> broadcast tile (parallel chain)
                                rv_sb = p2.tile([1, CH], f32r, tag="rv_sb", bufs=1, name="rv_sb")
                                nc.vector.reciprocal(rv_sb[:], rs_ps[0:1, :])
                                bc2_sb = p2.tile([128, CH], f32r, tag="bc2_sb", bufs=1, name="bc2_sb")
                                nc.gpsimd.partition_broadcast(bc2_sb[:], rv_sb[:])
                                # v_b expansion on unnormalized out_lat; normalize once on
                                # v_out (per-column scaling commutes with the contraction)
                                vo_ps = ps_tile("vo_ps")
                                for ci in range(4):
                                    nc.tensor.matmul(vo_ps[:], r(vb_sb[:, h, ci, :]), r(ol_sb[:, ci, :]),
                                                     start=(ci == 0), stop=(ci == 3))
                                nc.vector.tensor_mul(vo_sb[:, h, :], vo_ps[:], bc2_sb[:])
                            return tail

                        prev_tail = make_tail(h, ol_ps, rs_box[0])
                    prev_tail()

                    # project next chunk's q_nope before o_proj so its PSUM banks
                    # allocate ahead of oo_ps in the bank FIFO
                    if j + 1 < NCH:
                        qn_ps_cur = [ps_tile(f"qn_ps{h}") for h in range(HL)]
                        for kg in range(KT // 2):
                            for h in range(HL):
                                nc.tensor.matmul(qn_ps_cur[h][:],
                                                 wqn8_sb[:, kg, :, h * 128:(h + 1) * 128],
                                                 ht8c[:, kg], start=(kg == 0),
                                                 stop=(kg == KT // 2 - 1), perf_mode=DR2)

                    # o_proj partial: out^T[hid, s] = sum_h wo^T.T @ v_out^T
                    for htile in range(KT):
                        wo_sb = p2.tile([128, HL, 128], bf16, tag="wo_sb", bufs=6, name="wo_sb")
                        nc.sync.dma_start(wo_sb[:], wo_d[:, htile * 128:(htile + 1) * 128]
                                          .rearrange("(a p) n -> p a n", p=128))
                        oo_ps = ps_tile("oo_ps")
                        for hh2 in range(HL):
                            nc.tensor.matmul(oo_ps[:], wo_sb[:, hh2, :], vo_sb[:, hh2, :],
                                             start=(hh2 == 0), stop=(hh2 == HL - 1))
                        oo_sb = p2.tile([128, CH], bf16, tag="oo_sb", bufs=3, name="oo_sb")
                        nc.vector.tensor_copy(oo_sb[:], oo_ps[:])
                        nc.scalar.dma_start(out_d[htile * 128:(htile + 1) * 128, sl], oo_sb[:])

    nc.compile()
    return nc


# ---------------------------------------------------------------------------
# host-side input prep / output assembly
# ---------------------------------------------------------------------------
_PERM = np.concatenate([np.arange(0, DR, 2), np.arange(1, DR, 2)])


def _rope_tables(pos, s):
    inv_freq = 1.0 / (THETA ** (np.arange(0, DR, 2, dtype=np.float64) / DR))
    t = pos.astype(np.float64)
    freqs = t[:, None] * inv_freq
    emb = np.concatenate([freqs, freqs], axis=-1)          # [s, DR]
    cosT = np.cos(emb).T.astype(np.float32)                # [DR, s]
    sinT = np.sin(emb).T.astype(np.float32)
    cos_p = np.ascontiguousarray(np.vstack([cosT, cosT]))  # [128, s]
    sin_p = np.ascontiguousarray(np.vstack([sinT, sinT]))
    return cos_p, sin_p


def _masks():
    t = np.arange(128)[:, None]
    c = np.arange(CH)[None, :]
    m = np.zeros((128, 4, CH), np.float32)
    for kd in range(4):
        m[:, kd, :] = np.where(c >= 128 * kd + t, 0.0, -1e30).astype(np.float32)
    return m


def prep_core_inputs(inputs, core, s=S, hid=HID):
    import ml_dtypes

    b, g = core // 4, core % 4
    heads = slice(HL * g, HL * (g + 1))
    hs = np.asarray(inputs["hidden_states"], np.float32)[b, :s, :hid]
    hid_t = np.ascontiguousarray(hs.T)
    m = {"hid_t": hid_t.astype(ml_dtypes.bfloat16),
         "hid8_t": hid_t.astype(ml_dtypes.float8_e4m3)}

    # scale 0.02-std weights out of e4m3's subnormal range; 1/64 folds into k_b
    wq = np.asarray(inputs["q_nope_weight"], np.float32).reshape(H, DN, HID)[heads, :, :hid]
    wq_t = wq.transpose(2, 0, 1).reshape(hid, HL * DN)
    m["wqn8_t"] = np.ascontiguousarray(wq_t * 64.0).astype(ml_dtypes.float8_e4m3)

    wqp = np.asarray(inputs["q_pe_weight"], np.float32).reshape(H, DR, HID)[heads, :, :hid]
    a = wqp[:, _PERM, :]                                   # [4, 64, hid]
    bv = np.concatenate([-a[:, 32:64], a[:, 0:32]], axis=1)
    A = a.reshape(2, 128, hid)
    Bv = bv.reshape(2, 128, hid)
    wqpe_t = np.concatenate([A[0], A[1]], axis=0).T
    m["wqp_t"] = np.ascontiguousarray(wqpe_t).astype(ml_dtypes.bfloat16)

    wkv = np.asarray(inputs["kv_a_weight"], np.float32)[:, :hid]
    kpe_a = wkv[KVR:][_PERM]
    kpe_b = np.concatenate([-kpe_a[32:], kpe_a[:32]], axis=0)
    m["wkv_t"] = np.ascontiguousarray(
        np.concatenate([wkv[:KVR], kpe_a, kpe_b], axis=0).T).astype(ml_dtypes.bfloat16)

    m["ln_t"] = np.ascontiguousarray(
        np.asarray(inputs["kv_a_ln_weight"], np.float32).reshape(4, 128).T)
    m["kb"] = np.ascontiguousarray(
        np.asarray(inputs["k_b_weight"], np.float32)[heads] / 64.0)
    # ckvN on-device omits the ln weight; fold it into v_b's KVR axis instead
    ln_w = np.asarray(inputs["kv_a_ln_weight"], np.float32)
    m["vb_t"] = np.ascontiguousarray(
        np.asarray(inputs["v_b_weight"], np.float32)[heads].transpose(0, 2, 1)
        * ln_w[None, :, None])
    m["wo_t"] = np.ascontiguousarray(
        np.asarray(inputs["o_weight"], np.float32)[:hid, HL * DV * g:HL * DV * (g + 1)].T
    ).astype(ml_dtypes.bfloat16)

    pos = np.asarray(inputs["position_ids"]).reshape(-1)[:s]
    cos_p, sin_p = _rope_tables(pos, s)
    m["cos_p"], m["sin_p"] = cos_p, sin_p
    m["masks"] = _masks()
    m["ident"] = np.eye(128, dtype=np.float32)
    m["ones_c"] = np.ones((128, 1), np.float32)
    m["ones_r"] = np.ones((1, 128), np.float32)
    return m


_NC_CACHE = {}


def _get_nc():
    if "nc" not in _NC_CACHE:
        _NC_CACHE["nc"] = build_nc()
    return _NC_CACHE["nc"]


def kernel(**inputs):
    from concourse import bass_utils

    nc = _get_nc()
    in_maps = [prep_core_inputs(inputs, c) for c in range(NCORES)]
    res = bass_utils.run_bass_kernel_spmd(nc, in_maps, core_ids=list(range(NCORES)))
    out = np.empty((B, S, HID), np.float32)
    for b in range(B):
        acc = np.array(res.results[4 * b]["out_t"], np.float32)
        for g in range(1, 4):
            acc += res.results[4 * b + g]["out_t"]
        out[b] = acc.T
    return out

